# revision 14
# baseline (speedup 1.0000x reference)
"""Trainium2 Bass kernel for nn_LiteTransformer (sparse_attention).

Sharding (8 cores):
  - position-attention (down & up): by head (core c owns head c)
  - self-attention blocks: core c owns batch c//2, heads c%2*4..+4
  - decoder: token-sharded (2048 tokens per core)

Host->device transfer is the wall-clock bottleneck (axon tunnel ~120MB/s,
~0.2s per tensor latency), so inputs are packed into THREE small tensors
per core (~1.5MB total) instead of replicating the 16MB masked distance
matrices everywhere:
  - blob  (525,1024) bf16: m_cross row-shard + encoder-input slice +
    masked-percentile thresholds; AllGathered on device.
  - sheet (224,1024) bf16: 1/8 shard of all weights; AllGathered on device.
  - blobS (128,36)  f32 : per-head exp scales + biases (replicated).
The masked distance matrices exp(-c*(mc + big*(mc>thr))) are computed
on-device as exp(-c*mc) * (mc <= thr); thresholds are host-refined so the
bf16 comparison reproduces the exact f32 percentile mask.
"""

import numpy as np
import ml_dtypes

import jax
# run_bass_kernel_spmd builds a fresh jit closure per call; persist the XLA
# executable so repeat calls skip the ~0.5s re-compile (NEFF is already
# disk-cached separately).
jax.config.update("jax_compilation_cache_dir", "/tmp/jax_cache_kernel")
jax.config.update("jax_persistent_cache_min_entry_size_bytes", 0)
jax.config.update("jax_persistent_cache_min_compile_time_secs", 0)

import concourse.bass as bass
import concourse.mybir as mybir
import concourse.tile as tile
from concourse import bacc
from concourse.bass import ds
from concourse.bass_utils import run_bass_kernel_spmd
from concourse.masks import make_identity

BF = mybir.dt.bfloat16
F32 = mybir.dt.float32
AF = mybir.ActivationFunctionType
OP = mybir.AluOpType
NPBF = ml_dtypes.bfloat16

B, RES, N, M, H, D, KD, NB = 4, 64, 4096, 1024, 8, 256, 32, 4
BN = B * N
NCORE = 8
INV_SQRT_K = float(1.0 / np.sqrt(np.float32(KD)))
ALL8 = [list(range(NCORE))]
PAIRS = [[0, 1], [2, 3], [4, 5], [6, 7]]

# blob layout (per-core rows, width 1024 bf16)
BLOB_ROWS = 525          # 512 mc + 8 enc + 1 t_down + 4 t_up
R_ENC = 512
R_TDOWN = 520
R_TUP = 521
# sheet layout (global rows, width 1024 bf16)
SHEET_ROWS = 1792        # 224 per core
SH_W1, SH_W2, SH_WR = 0, 256, 512
SH_QP, SH_KP, SH_VP = 768, 1024, 1280
SH_MISC = 1536           # cols 0:256 wde1 | 256:512 wdown | 512:768 wup | 768: misc2
# misc2: rows SH_MISC..+4 cols 768:1024 = wen; wde2 halves at cols 770,771 rows +8..+136
# blobS cols
SC_NCD, SC_NCU, SC_BEN, SC_B1, SC_BC, SC_BD1, SCOLS = 0, 8, 16, 18, 26, 34, 36

_cache = {}


def _build():
    nc = bacc.Bacc("TRN2", target_bir_lowering=False, debug=False,
                   num_devices=NCORE)

    blob = nc.dram_tensor("blob", [BLOB_ROWS, 1024], BF,
                          kind="ExternalInput").ap()
    sheet = nc.dram_tensor("sheet", [SHEET_ROWS // NCORE, 1024], BF,
                           kind="ExternalInput").ap()
    blobS = nc.dram_tensor("blobS", [128, SCOLS], F32,
                           kind="ExternalInput").ap()
    out_shard = nc.dram_tensor("out_shard", [1, BN // NCORE], F32,
                               kind="ExternalOutput").ap()

    with tile.TileContext(nc) as tc:
        with (
            tc.tile_pool(name="dram", bufs=1, space="DRAM") as dram,
            tc.tile_pool(name="consts", bufs=1) as consts,
            tc.tile_pool(name="small", bufs=6) as small,
            tc.tile_pool(name="pp", bufs=4, space="PSUM") as pp,
            tc.tile_pool(name="pt", bufs=2, space="PSUM") as ppt,
        ):
            ident = consts.tile([128, 128], BF, name="ident", tag="ident")
            make_identity(nc, ident)
            pid = nc.sync.partition_id()

            # ---- gather the packed inputs across cores ----
            # (collectives cannot read IO tensors; bounce through DRAM scratch)
            blob_in = dram.tile([BLOB_ROWS, 1024], BF, name="blobi",
                                tag="blobi")
            nc.sync.dma_start(blob_in[:, :], blob[:, :])
            blob_out = dram.tile([NCORE * BLOB_ROWS, 1024], BF, name="blobo",
                                 tag="blobo", addr_space="Shared")
            nc.gpsimd.collective_compute(
                "AllGather", OP.bypass, replica_groups=ALL8,
                ins=[blob_in.opt()], outs=[blob_out.opt()])
            sheet_in = dram.tile([SHEET_ROWS // NCORE, 1024], BF,
                                 name="sheeti", tag="sheeti")
            nc.sync.dma_start(sheet_in[:, :], sheet[:, :])
            sheet_out = dram.tile([SHEET_ROWS, 1024], BF, name="sheeto",
                                  tag="sheeto", addr_space="Shared")
            nc.gpsimd.collective_compute(
                "AllGather", OP.bypass, replica_groups=ALL8,
                ins=[sheet_in.opt()], outs=[sheet_out.opt()])

            # clean (4096, 1024) m_cross scratch out of the gathered chunks
            mc_dram = dram.tile([N, 1024], BF, name="mcd", tag="mcd")
            for c in range(NCORE):
                nc.sync.dma_start(
                    mc_dram[c * 512:(c + 1) * 512, :],
                    blob_out[c * BLOB_ROWS:c * BLOB_ROWS + 512, :])

            blobS_sb = consts.tile([128, SCOLS], F32, name="bS", tag="bS")
            nc.sync.dma_start(blobS_sb[:], blobS[:, :])
            ncd_sb = consts.tile([128, 1], F32, name="ncd", tag="ncd")
            nc.sync.dma_start(ncd_sb[:], blobS[0:128, ds(SC_NCD + pid, 1)])
            ncu_sb = consts.tile([128, 1], F32, name="ncu", tag="ncu")
            nc.sync.dma_start(ncu_sb[:], blobS[0:128, ds(SC_NCU + pid, 1)])

            # thresholds broadcast to 128 partitions (ones ⊗ row via matmul)
            ones_sb = consts.tile([1, 128], BF, name="ones", tag="ones")
            nc.vector.memset(ones_sb[:], 1.0)
            td_row = consts.tile([1, 1024], BF, name="tdr", tag="tdr")
            nc.sync.dma_start(td_row[:], blob_out[R_TDOWN:R_TDOWN + 1, :])
            thrD = consts.tile([128, 1024], BF, name="thrD", tag="thrD")
            for hf in range(2):
                pb = pp.tile([128, 512], F32, name="pp", tag="pp")
                nc.tensor.matmul(pb[:], ones_sb[:],
                                 td_row[:, hf * 512:(hf + 1) * 512])
                nc.vector.tensor_copy(thrD[:, hf * 512:(hf + 1) * 512], pb[:])

            wen_sb = consts.tile([4, 256], BF, name="wen", tag="wen")
            nc.sync.dma_start(wen_sb[:], sheet_out[SH_MISC:SH_MISC + 4, 768:1024])
            wdn_sb = []
            wup_sb = []
            for t in range(2):
                w = consts.tile([128, KD], BF, name=f"wdn{t}", tag=f"wdn{t}")
                nc.sync.dma_start(
                    w[:], sheet_out[SH_MISC + t * 128:SH_MISC + (t + 1) * 128,
                                    ds(256 + pid * KD, KD)])
                wdn_sb.append(w)
                w = consts.tile([128, KD], BF, name=f"wupt{t}", tag=f"wupt{t}")
                nc.sync.dma_start(
                    w[:], sheet_out[SH_MISC + t * 128:SH_MISC + (t + 1) * 128,
                                    ds(512 + pid * KD, KD)])
                wup_sb.append(w)
            ben_sb = [blobS_sb[:, SC_BEN + t:SC_BEN + t + 1] for t in range(2)]

            TS = BN // NCORE  # 2048
            # ag1/ag4 use AllToAll (each core only needs its batch / token
            # slice), which shrinks the wire bytes 4-8x vs AllGather.
            ag1_in = dram.tile([NCORE * 32, M], BF, name="ag1i", tag="ag1i")
            ag1_out = dram.tile([NCORE * 32, M], BF, name="ag1o", tag="ag1o")
            ag3_in = dram.tile([D, M], BF, name="ag3i", tag="ag3i")
            ag3_out = dram.tile([NCORE * D, M], BF, name="ag3o", tag="ag3o",
                                addr_space="Shared")
            ag4_in = dram.tile([NCORE * KD, TS], BF, name="ag4i", tag="ag4i")
            ag4_out = dram.tile([NCORE * KD, TS], BF, name="ag4o", tag="ag4o")

            def psum(p, f, dt=F32):
                return pp.tile([p, f], dt, name="pp", tag="pp")

            _lwn = [0]

            def lw(pool, p0, p1, f0, f1, dt=BF):
                # load a (p1-p0, f1-f0) tile from sheet_out
                _lwn[0] += 1
                t = pool.tile([p1 - p0, f1 - f0], dt, name=f"lw{_lwn[0]}",
                              tag=f"lw{_lwn[0]}")
                nc.sync.dma_start(t[:], sheet_out[p0:p1, f0:f1])
                return t

            # ---------------- P1: down (head pid over full N) ----------------
            with tc.tile_pool(name="p1", bufs=3) as p1, \
                 tc.tile_pool(name="p1keep", bufs=1) as p1k:
                # encoder input (4, BN) from the 16 gathered groups
                enc_sb = p1k.tile([4, BN], BF, name="enc_sb", tag="enc_sb")
                for g in range(16):
                    src = (g // 2) * BLOB_ROWS + R_ENC + (g % 2) * 4
                    nc.sync.dma_start(enc_sb[:, g * 1024:(g + 1) * 1024],
                                      blob_out[src:src + 4, :])
                v_all = [p1k.tile([128, 4 * 33], BF, name=f"va{i}", tag=f"va{i}")
                         for i in range(32)]
                for b in range(B):
                    for ni in range(32):
                        off = b * N + ni * 128
                        enT = []
                        for t in range(2):
                            pe = psum(128, 128)
                            nc.tensor.matmul(
                                pe[:], wen_sb[:, t * 128:(t + 1) * 128],
                                enc_sb[:, off:off + 128])
                            g = p1.tile([128, 128], BF, name="enT", tag="enT")
                            nc.scalar.activation(g[:], pe[:], AF.Gelu,
                                                 bias=ben_sb[t])
                            enT.append(g)
                        pv = psum(128, KD)
                        for t in range(2):
                            nc.tensor.matmul(pv[:], enT[t][:], wdn_sb[t][:],
                                             start=(t == 0), stop=(t == 1))
                        nc.vector.tensor_copy(
                            v_all[ni][:, b * 33:b * 33 + KD], pv[:])
                        if b == 0:
                            for bb in range(B):
                                nc.vector.memset(
                                    v_all[ni][:, bb * 33 + 32:bb * 33 + 33],
                                    1.0)

                # a_sb[ni] = exp(-c_h * mc) * (mc <= thr)
                a_sb = [p1k.tile([128, M], BF, name=f"as{i}", tag=f"as{i}")
                        for i in range(32)]
                for ni in range(32):
                    mct = p1.tile([128, M], BF, name="mct", tag="mct")
                    nc.sync.dma_start(mct[:],
                                      mc_dram[ni * 128:(ni + 1) * 128, :])
                    msk = p1.tile([128, M], BF, name="msk", tag="msk")
                    nc.vector.tensor_tensor(msk[:], mct[:], thrD[:], OP.is_le)
                    nc.scalar.activation(a_sb[ni][:], mct[:], AF.Exp,
                                         scale=ncd_sb[:])
                    nc.vector.tensor_tensor(a_sb[ni][:], a_sb[ni][:], msk[:],
                                            OP.mult)

                xhT = p1k.tile([128, M], BF, name="xhT", tag="xhT")
                for mi in range(8):
                    px = psum(128, 4 * 33)
                    for ni in range(32):
                        nc.tensor.matmul(
                            px[:], a_sb[ni][:, mi * 128:(mi + 1) * 128],
                            v_all[ni][:], start=(ni == 0), stop=(ni == 31))
                    for b in range(B):
                        rc = small.tile([128, 1], F32, name="rc", tag="rc")
                        nc.vector.reciprocal(
                            rc[:], px[:, b * 33 + 32:b * 33 + 33])
                        gx = small.tile([128, KD], BF, name="gx", tag="gx")
                        nc.scalar.activation(gx[:], px[:, b * 33:b * 33 + KD],
                                             AF.Gelu, scale=rc[:])
                        ptr = ppt.tile([KD, 128], BF, name="tp", tag="tp")
                        nc.tensor.transpose(ptr[:], gx[:], ident[:])
                        nc.vector.tensor_copy(
                            xhT[b * 32:b * 32 + 32,
                                mi * 128:(mi + 1) * 128], ptr[:])
                # core h sends batch c//2's 32 feature rows to core c
                for c in range(NCORE):
                    nc.sync.dma_start(
                        ag1_in[c * 32:(c + 1) * 32, :],
                        xhT[(c // 2) * 32:(c // 2) * 32 + 32, :])
                nc.gpsimd.collective_compute(
                    "AllToAll", OP.bypass, replica_groups=ALL8,
                    ins=[ag1_in.opt()], outs=[ag1_out.opt()])

            # ---------------- P2: blocks ----------------
            b0x32 = (pid // 2) * 32
            # static range clamp only — the emitted runtime-assert sequencer
            # instruction faults the exec unit on this runtime, so skip it
            hcol = nc.s_assert_within((pid - (pid // 2) * 2) * 128, 0, 128,
                                      skip_runtime_assert=True)
            with tc.tile_pool(name="p2", bufs=2) as p2, \
                 tc.tile_pool(name="p2e", bufs=1) as p2e:
                xT = [p2e.tile([128, M], BF, name=f"xT{t}", tag=f"xT{t}")
                      for t in range(2)]
                for hh in range(H):
                    nc.sync.dma_start(
                        xT[hh // 4][(hh % 4) * 32:(hh % 4) * 32 + 32, :],
                        ag1_out[hh * 32:(hh + 1) * 32, :])

                for blk in range(NB):
                    qp_sb, kp_sb, vp_sb = [], [], []
                    for t in range(2):
                        for dst, base in ((qp_sb, SH_QP), (kp_sb, SH_KP),
                                          (vp_sb, SH_VP)):
                            _lwn[0] += 1
                            w = p2.tile([128, 128], BF, name=f"lw{_lwn[0]}",
                                        tag=f"lw{_lwn[0]}")
                            nc.sync.dma_start(
                                w[:], sheet_out[base + t * 128:
                                                base + (t + 1) * 128,
                                                ds(blk * 256 + hcol, 128)])
                            dst.append(w)

                    qt = p2e.tile([128, M], BF, name="qt", tag="qt")
                    kt = p2e.tile([128, M], BF, name="kt", tag="kt")
                    for dst, wsb in ((qt, qp_sb), (kt, kp_sb)):
                        for mh in range(2):
                            pq = psum(128, 512)
                            for t in range(2):
                                nc.tensor.matmul(
                                    pq[:], wsb[t][:],
                                    xT[t][:, mh * 512:(mh + 1) * 512],
                                    start=(t == 0), stop=(t == 1))
                            nc.vector.tensor_copy(
                                dst[:, mh * 512:(mh + 1) * 512], pq[:])
                    qh = [p2e.tile([KD, M], BF, name=f"qh{h}", tag=f"qh{h}")
                          for h in range(4)]
                    kh = [p2e.tile([KD, M], BF, name=f"kh{h}", tag=f"kh{h}")
                          for h in range(4)]
                    for h in range(4):
                        nc.vector.tensor_copy(qh[h][:],
                                              qt[h * 32:h * 32 + 32, :])
                        nc.vector.tensor_copy(kh[h][:],
                                              kt[h * 32:h * 32 + 32, :])

                    vh = [p2e.tile([128, 4 * 33], BF, name=f"vh{ni}",
                                   tag=f"vh{ni}") for ni in range(8)]
                    for ni in range(8):
                        pvv = psum(128, 128)
                        for t in range(2):
                            nc.tensor.matmul(
                                pvv[:],
                                xT[t][:, ni * 128:(ni + 1) * 128],
                                vp_sb[t][:], start=(t == 0), stop=(t == 1))
                        for h in range(4):
                            nc.vector.tensor_copy(
                                vh[ni][:, h * 33:h * 33 + KD],
                                pvv[:, h * 32:h * 32 + 32])
                            nc.vector.memset(
                                vh[ni][:, h * 33 + 32:h * 33 + 33], 1.0)

                    es = [[p2e.tile([128, M], BF, name=f"es{h}_{ni}",
                                    tag=f"es{h}_{ni}")
                           for ni in range(8)] for h in range(4)]
                    for h in range(4):
                        for ni in range(8):
                            for mh in range(2):
                                psc = psum(128, 512)
                                nc.tensor.matmul(
                                    psc[:],
                                    kh[h][:, ni * 128:(ni + 1) * 128],
                                    qh[h][:, mh * 512:(mh + 1) * 512])
                                nc.scalar.activation(
                                    es[h][ni][:, mh * 512:(mh + 1) * 512],
                                    psc[:], AF.Exp, scale=INV_SQRT_K)

                    paT = p2e.tile([128, M], BF, name="paT", tag="paT")
                    for mi in range(8):
                        for h in range(4):
                            pa = psum(128, 33)
                            for ni in range(8):
                                nc.tensor.matmul(
                                    pa[:],
                                    es[h][ni][:, mi * 128:(mi + 1) * 128],
                                    vh[ni][:, h * 33:h * 33 + 33],
                                    start=(ni == 0), stop=(ni == 7))
                            rc = small.tile([128, 1], F32, name="rc", tag="rc")
                            nc.vector.reciprocal(rc[:], pa[:, 32:33])
                            gx = small.tile([128, KD], BF, name="gx", tag="gx")
                            nc.scalar.activation(gx[:], pa[:, 0:KD], AF.Gelu,
                                                 scale=rc[:])
                            ptr = ppt.tile([KD, 128], BF, name="tp", tag="tp")
                            nc.tensor.transpose(ptr[:], gx[:], ident[:])
                            nc.vector.tensor_copy(
                                paT[h * 32:h * 32 + 32,
                                    mi * 128:(mi + 1) * 128], ptr[:])

                    ag2_in = dram.tile([128, M], BF, name="ag2i", tag="ag2i")
                    ag2_out = dram.tile([D, M], BF, name="ag2o", tag="ag2o")
                    nc.sync.dma_start(ag2_in[:], paT[:])
                    nc.gpsimd.collective_compute(
                        "AllGather", OP.bypass,
                        replica_groups=PAIRS,
                        ins=[ag2_in.opt()], outs=[ag2_out.opt()])
                    paF = [p2e.tile([128, M], BF, name=f"paF{t}", tag=f"paF{t}")
                           for t in range(2)]
                    for t in range(2):
                        nc.sync.dma_start(
                            paF[t][:], ag2_out[t * 128:(t + 1) * 128, :])

                    w1_sb = [[lw(p2, SH_W1 + i * 128, SH_W1 + (i + 1) * 128,
                                 blk * 256 + o * 128, blk * 256 + (o + 1) * 128)
                              for o in range(2)] for i in range(2)]
                    b1_sb = [blobS_sb[:, SC_B1 + blk * 2 + t:
                                      SC_B1 + blk * 2 + t + 1]
                             for t in range(2)]
                    h1 = [p2e.tile([128, M], BF, name=f"h1{t}", tag=f"h1{t}")
                          for t in range(2)]
                    for o in range(2):
                        for mh in range(2):
                            ph = psum(128, 512)
                            for i in range(2):
                                nc.tensor.matmul(
                                    ph[:], w1_sb[i][o][:],
                                    paF[i][:, mh * 512:(mh + 1) * 512],
                                    start=(i == 0), stop=(i == 1))
                            nc.scalar.activation(
                                h1[o][:, mh * 512:(mh + 1) * 512], ph[:],
                                AF.Gelu, bias=b1_sb[o])

                    w2_sb = [[lw(p2, SH_W2 + i * 128, SH_W2 + (i + 1) * 128,
                                 blk * 256 + o * 128, blk * 256 + (o + 1) * 128)
                              for o in range(2)] for i in range(2)]
                    wr_sb = [[lw(p2, SH_WR + i * 128, SH_WR + (i + 1) * 128,
                                 blk * 256 + o * 128, blk * 256 + (o + 1) * 128)
                              for o in range(2)] for i in range(2)]
                    bc_sb = [blobS_sb[:, SC_BC + blk * 2 + t:
                                      SC_BC + blk * 2 + t + 1]
                             for t in range(2)]
                    xn = [p2e.tile([128, M], BF, name=f"xn{t}", tag=f"xn{t}")
                          for t in range(2)]
                    for o in range(2):
                        for mh in range(2):
                            po = psum(128, 512)
                            nc.tensor.matmul(
                                po[:], w2_sb[0][o][:],
                                h1[0][:, mh * 512:(mh + 1) * 512],
                                start=True, stop=False)
                            nc.tensor.matmul(
                                po[:], w2_sb[1][o][:],
                                h1[1][:, mh * 512:(mh + 1) * 512],
                                start=False, stop=False)
                            nc.tensor.matmul(
                                po[:], wr_sb[0][o][:],
                                xT[0][:, mh * 512:(mh + 1) * 512],
                                start=False, stop=False)
                            nc.tensor.matmul(
                                po[:], wr_sb[1][o][:],
                                xT[1][:, mh * 512:(mh + 1) * 512],
                                start=False, stop=True)
                            nc.scalar.activation(
                                xn[o][:, mh * 512:(mh + 1) * 512], po[:],
                                AF.Gelu, bias=bc_sb[o])
                    xT = xn

                for t in range(2):
                    nc.sync.dma_start(ag3_in[t * 128:(t + 1) * 128, :],
                                      xT[t][:])
                nc.gpsimd.collective_compute(
                    "AllGather", OP.bypass, replica_groups=ALL8,
                    ins=[ag3_in.opt()], outs=[ag3_out.opt()])

            # ---------------- P4: up (head pid over full N) ----------------
            with tc.tile_pool(name="p4", bufs=3) as p4, \
                 tc.tile_pool(name="p4keep", bufs=1) as p4k:
                vu = [p4k.tile([128, 4 * 33], BF, name=f"vu{i}", tag=f"vu{i}")
                      for i in range(8)]
                for b in range(B):
                    xb = [p4.tile([128, M], BF, name=f"xb{t}", tag=f"xb{t}")
                          for t in range(2)]
                    for t in range(2):
                        nc.sync.dma_start(
                            xb[t][:],
                            ag3_out[2 * b * D + t * 128:
                                    2 * b * D + (t + 1) * 128, :])
                    for mi in range(8):
                        pv = psum(128, KD)
                        for t in range(2):
                            nc.tensor.matmul(
                                pv[:], xb[t][:, mi * 128:(mi + 1) * 128],
                                wup_sb[t][:], start=(t == 0), stop=(t == 1))
                        nc.vector.tensor_copy(
                            vu[mi][:, b * 33:b * 33 + KD], pv[:])
                        if b == 0:
                            for bb in range(B):
                                nc.vector.memset(
                                    vu[mi][:, bb * 33 + 32:bb * 33 + 33], 1.0)

                # thrU (128, N) from the gathered t_up rows (matmul broadcast)
                tu_row = p4k.tile([1, N], BF, name="tur", tag="tur")
                for r in range(4):
                    nc.sync.dma_start(
                        tu_row[:, r * 1024:(r + 1) * 1024],
                        blob_out[R_TUP + r:R_TUP + r + 1, :])
                thrU = p4k.tile([128, N], BF, name="thrU", tag="thrU")
                for hf in range(8):
                    pb = pp.tile([128, 512], F32, name="pp", tag="pp")
                    nc.tensor.matmul(pb[:], ones_sb[:],
                                     tu_row[:, hf * 512:(hf + 1) * 512])
                    nc.vector.tensor_copy(thrU[:, hf * 512:(hf + 1) * 512],
                                          pb[:])

                # eu[ki] = exp(-c_h * mcT) * (mcT <= thrU), mcT via DMA transpose
                eu = [p4k.tile([128, N], BF, name=f"eu{i}", tag=f"eu{i}")
                      for i in range(8)]
                for ki in range(8):
                    mctT = p4.tile([128, N], BF, name="mctT", tag="mctT")
                    nc.sync.dma_start(mctT[:],
                                      mc_dram[:, ki * 128:(ki + 1) * 128],
                                      transpose=True)
                    msk = p4.tile([128, N], BF, name="umsk", tag="umsk")
                    nc.vector.tensor_tensor(msk[:], mctT[:], thrU[:], OP.is_le)
                    nc.scalar.activation(eu[ki][:], mctT[:], AF.Exp,
                                         scale=ncu_sb[:])
                    nc.vector.tensor_tensor(eu[ki][:], eu[ki][:], msk[:],
                                            OP.mult)

                deT = p4k.tile([KD, BN], BF, name="deT", tag="deT")
                for qi in range(32):
                    pd = psum(128, 4 * 33)
                    for ki in range(8):
                        nc.tensor.matmul(
                            pd[:], eu[ki][:, qi * 128:(qi + 1) * 128],
                            vu[ki][:], start=(ki == 0), stop=(ki == 7))
                    for b in range(B):
                        rc = small.tile([128, 1], F32, name="rc", tag="rc")
                        nc.vector.reciprocal(
                            rc[:], pd[:, b * 33 + 32:b * 33 + 33])
                        gx = small.tile([128, KD], BF, name="gx", tag="gx")
                        nc.scalar.activation(gx[:], pd[:, b * 33:b * 33 + KD],
                                             AF.Gelu, scale=rc[:])
                        ptr = ppt.tile([KD, 128], BF, name="tp", tag="tp")
                        nc.tensor.transpose(ptr[:], gx[:], ident[:])
                        nc.vector.tensor_copy(
                            deT[:, b * N + qi * 128:b * N + (qi + 1) * 128],
                            ptr[:])
                # core h sends token chunk c (its 32 de^T rows) to core c
                for c in range(NCORE):
                    nc.sync.dma_start(ag4_in[c * 32:(c + 1) * 32, :],
                                      deT[:, c * TS:(c + 1) * TS])
                nc.gpsimd.collective_compute(
                    "AllToAll", OP.bypass, replica_groups=ALL8,
                    ins=[ag4_in.opt()], outs=[ag4_out.opt()])

            # ---------------- P5: decoder on token shard ----------------
            with tc.tile_pool(name="p5", bufs=2) as p5:
                dea = [p5.tile([128, TS], BF, name=f"dea{t}", tag=f"dea{t}")
                       for t in range(2)]
                for t in range(2):
                    nc.sync.dma_start(
                        dea[t][:],
                        ag4_out[t * 128:(t + 1) * 128, :])
                wd1 = [[lw(p5, SH_MISC + i * 128, SH_MISC + (i + 1) * 128,
                           o * 128, (o + 1) * 128) for o in range(2)]
                       for i in range(2)]
                bd1 = [blobS_sb[:, SC_BD1 + t:SC_BD1 + t + 1]
                       for t in range(2)]
                wd2 = [lw(p5, SH_MISC + 8, SH_MISC + 8 + 128,
                          770 + t, 771 + t) for t in range(2)]
                g = [p5.tile([128, TS], BF, name=f"g{t}", tag=f"g{t}")
                     for t in range(2)]
                for o in range(2):
                    for th in range(4):
                        pg = psum(128, 512)
                        for i in range(2):
                            nc.tensor.matmul(
                                pg[:], wd1[i][o][:],
                                dea[i][:, th * 512:(th + 1) * 512],
                                start=(i == 0), stop=(i == 1))
                        nc.scalar.activation(
                            g[o][:, th * 512:(th + 1) * 512], pg[:],
                            AF.Gelu, bias=bd1[o])
                osb = p5.tile([1, TS], F32, name="osb", tag="osb")
                for th in range(4):
                    p2o = psum(1, 512)
                    for i in range(2):
                        nc.tensor.matmul(
                            p2o[:], wd2[i][:],
                            g[i][:, th * 512:(th + 1) * 512],
                            start=(i == 0), stop=(i == 1))
                    nc.vector.tensor_copy(
                        osb[:, th * 512:(th + 1) * 512], p2o[:])
                nc.sync.dma_start(out_shard[:, :], osb[:])

    nc.compile()
    return nc


def _prep_inputs(inputs, m_cross, W_en, b_en, r_down, w_down, q_pa, k_pa,
                 v_pa, W1_mlp, b1_mlp, W2_mlp, b2_mlp, W_res, b_res, r_up,
                 w_up, W_de1, b_de1, W_de2, b_de2, y_mean, y_std):
    f32 = np.float32
    mc = np.asarray(m_cross, f32)
    mcb = mc.astype(NPBF)
    mcbf = mcb.astype(f32)

    # bf16-robust percentile thresholds: max passing bf16 value so the
    # device-side (bf16 mc <= thr) comparison reproduces the f32 mask
    kd_ = int(0.30 * (N - 1))          # 1228
    vkd = np.partition(mc, kd_, axis=0)[kd_, :]               # (M,)
    t_down_dev = np.where(mc <= vkd[None, :], mcbf, -np.inf).max(axis=0)
    ku_ = int(0.30 * (M - 1))          # 306
    vku = np.partition(mc, ku_, axis=1)[:, ku_]               # (N,)
    t_up_dev = np.where(mc <= vku[:, None], mcbf, -np.inf).max(axis=1)

    # encoder input (4, BN), group-major rows of 1024
    gx = np.linspace(0.0, 1.0, RES + 1, dtype=f32)[:-1]
    gxx = np.broadcast_to(gx[:, None], (RES, RES))
    gyy = np.broadcast_to(gx[None, :], (RES, RES))
    encf = np.zeros((4, BN), f32)
    encf[0, :] = np.tile(gxx.reshape(-1), B)
    encf[1, :] = np.tile(gyy.reshape(-1), B)
    encf[2, :] = np.asarray(inputs, f32).reshape(BN)
    enc_cat = encf.reshape(4, 16, 1024).transpose(1, 0, 2).reshape(64, 1024)

    c_down = np.tan(0.25 * np.pi * (1.0 + np.sin(np.asarray(r_down, f32)
                                                 .reshape(H)))).astype(f32)
    c_up = np.tan(0.25 * np.pi * (1.0 + np.sin(np.asarray(r_up, f32)
                                               .reshape(H)))).astype(f32)

    # ---- weight sheet (1792, 1024) ----
    sheet = np.zeros((SHEET_ROWS, 1024), NPBF)

    def cat_blocks(w):  # (NB, D, D) -> (D, NB*D)
        return np.asarray(w, f32).transpose(1, 0, 2).reshape(D, NB * D)

    def cat_heads(w):   # (NB, H, D, K) -> (D, NB*H*K)
        return np.asarray(w, f32).transpose(2, 0, 1, 3).reshape(D, NB * H * KD)

    sheet[SH_W1:SH_W1 + D] = cat_blocks(W1_mlp).astype(NPBF)
    sheet[SH_W2:SH_W2 + D] = cat_blocks(W2_mlp).astype(NPBF)
    sheet[SH_WR:SH_WR + D] = cat_blocks(W_res).astype(NPBF)
    sheet[SH_QP:SH_QP + D] = cat_heads(q_pa).astype(NPBF)
    sheet[SH_KP:SH_KP + D] = cat_heads(k_pa).astype(NPBF)
    sheet[SH_VP:SH_VP + D] = cat_heads(v_pa).astype(NPBF)
    sheet[SH_MISC:SH_MISC + D, 0:256] = np.asarray(W_de1, f32).astype(NPBF)
    sheet[SH_MISC:SH_MISC + D, 256:512] = (
        np.asarray(w_down, f32).transpose(1, 0, 2).reshape(D, H * KD)
        .astype(NPBF))
    sheet[SH_MISC:SH_MISC + D, 512:768] = (
        np.asarray(w_up, f32).transpose(1, 0, 2).reshape(D, H * KD)
        .astype(NPBF))
    wen4 = np.zeros((4, D), f32)
    wen4[:3, :] = np.asarray(W_en, f32)
    sheet[SH_MISC:SH_MISC + 4, 768:1024] = wen4.astype(NPBF)
    ystd = float(np.asarray(y_std, f32))
    ymean = float(np.asarray(y_mean, f32))
    wde2f = (np.asarray(W_de2, f32).reshape(D) * ystd).astype(NPBF)
    sheet[SH_MISC + 8:SH_MISC + 8 + 128, 770] = wde2f[0:128]
    sheet[SH_MISC + 8:SH_MISC + 8 + 128, 771] = wde2f[128:256]
    bde2f = float(np.asarray(b_de2, f32).reshape(-1)[0] * ystd + ymean)

    # ---- f32 constants (128, 36) ----
    bS = np.zeros((128, SCOLS), f32)
    bS[:, SC_NCD:SC_NCD + 8] = -c_down[None, :]
    bS[:, SC_NCU:SC_NCU + 8] = -c_up[None, :]
    ben = np.asarray(b_en, f32).reshape(D)
    bS[:, SC_BEN] = ben[0:128]
    bS[:, SC_BEN + 1] = ben[128:256]
    b1f = np.asarray(b1_mlp, f32).reshape(NB, D)
    bcf = (np.asarray(b2_mlp, f32) + np.asarray(b_res, f32)).reshape(NB, D)
    for blk in range(NB):
        for t in range(2):
            bS[:, SC_B1 + blk * 2 + t] = b1f[blk, t * 128:(t + 1) * 128]
            bS[:, SC_BC + blk * 2 + t] = bcf[blk, t * 128:(t + 1) * 128]
    bd1 = np.asarray(b_de1, f32).reshape(D)
    bS[:, SC_BD1] = bd1[0:128]
    bS[:, SC_BD1 + 1] = bd1[128:256]

    in_maps = []
    for c in range(NCORE):
        blob = np.zeros((BLOB_ROWS, 1024), NPBF)
        blob[0:512] = mcb[c * 512:(c + 1) * 512]
        blob[R_ENC:R_ENC + 8] = enc_cat[c * 8:(c + 1) * 8].astype(NPBF)
        blob[R_TDOWN] = t_down_dev.astype(NPBF)
        blob[R_TUP:R_TUP + 4] = t_up_dev.reshape(4, 1024).astype(NPBF)
        in_maps.append({
            "blob": blob,
            "sheet": sheet[c * (SHEET_ROWS // NCORE):
                           (c + 1) * (SHEET_ROWS // NCORE)],
            "blobS": bS,
        })
    return in_maps, bde2f


def kernel(**inputs):
    if "nc" not in _cache:
        _cache["nc"] = _build()
    nc = _cache["nc"]
    in_maps, bde2f = _prep_inputs(**inputs)
    res = run_bass_kernel_spmd(nc, in_maps, core_ids=list(range(NCORE)))
    shards = [res.results[c]["out_shard"].reshape(-1) + np.float32(bde2f)
              for c in range(NCORE)]
    out = np.concatenate(shards).astype(np.float32)
    return out.reshape(B, RES, RES, 1)


# revision 16
# speedup vs baseline: 1.6740x; 1.6740x over previous
"""Trainium2 Bass kernel for nn_LiteTransformer (sparse_attention).

Sharding (8 cores):
  - position-attention (down & up): by head (core c owns head c)
  - self-attention blocks: core c owns batch c//2, heads c%2*4..+4
  - decoder: token-sharded (2048 tokens per core)

Host->device transfer is the wall-clock bottleneck (axon tunnel ~120MB/s,
~0.2s per tensor latency), so inputs are packed into THREE small tensors
per core (~1.5MB total) instead of replicating the 16MB masked distance
matrices everywhere:
  - blob  (525,1024) bf16: m_cross row-shard + encoder-input slice +
    masked-percentile thresholds; AllGathered on device.
  - sheet (224,1024) bf16: 1/8 shard of all weights; AllGathered on device.
  - blobS (128,36)  f32 : per-head exp scales + biases (replicated).
The masked distance matrices exp(-c*(mc + big*(mc>thr))) are computed
on-device as exp(-c*mc) * (mc <= thr); thresholds are host-refined so the
bf16 comparison reproduces the exact f32 percentile mask.
"""

import numpy as np
import ml_dtypes

import jax
# run_bass_kernel_spmd builds a fresh jit closure per call; persist the XLA
# executable so repeat calls skip the ~0.5s re-compile (NEFF is already
# disk-cached separately).
jax.config.update("jax_compilation_cache_dir", "/tmp/jax_cache_kernel")
jax.config.update("jax_persistent_cache_min_entry_size_bytes", 0)
jax.config.update("jax_persistent_cache_min_compile_time_secs", 0)

import concourse.bass as bass
import concourse.mybir as mybir
import concourse.tile as tile
from concourse import bacc
from concourse.bass import ds
from concourse.bass_utils import (run_bass_kernel_spmd as _lib_run_spmd,
                                  BassKernelResults)
from concourse.masks import make_identity

BF = mybir.dt.bfloat16
F32 = mybir.dt.float32
AF = mybir.ActivationFunctionType
OP = mybir.AluOpType
NPBF = ml_dtypes.bfloat16

B, RES, N, M, H, D, KD, NB = 4, 64, 4096, 1024, 8, 256, 32, 4
BN = B * N
NCORE = 8
INV_SQRT_K = float(1.0 / np.sqrt(np.float32(KD)))
ALL8 = [list(range(NCORE))]
PAIRS = [[0, 1], [2, 3], [4, 5], [6, 7]]

# blob layout (per-core rows, width 1024 bf16)
BLOB_ROWS = 525          # 512 mc + 8 enc + 1 t_down + 4 t_up
R_ENC = 512
R_TDOWN = 520
R_TUP = 521
# sheet layout (global rows, width 1024 bf16)
SHEET_ROWS = 1792        # 224 per core
SH_W1, SH_W2, SH_WR = 0, 256, 512
SH_QP, SH_KP, SH_VP = 768, 1024, 1280
SH_MISC = 1536           # cols 0:256 wde1 | 256:512 wdown | 512:768 wup | 768: misc2
# misc2: rows SH_MISC..+4 cols 768:1024 = wen; wde2 halves at cols 770,771 rows +8..+136
# blobS cols
SC_NCD, SC_NCU, SC_BEN, SC_B1, SC_BC, SC_BD1, SCOLS = 0, 8, 16, 18, 26, 34, 36

_cache = {}
_exec_cache = {}


def run_bass_kernel_spmd(nc, in_maps, core_ids, **kw):
    """Same semantics as bass_utils.run_bass_kernel_spmd for the plain SPMD
    case, but keeps the jitted executable across calls (the library builds a
    fresh closure per call, costing ~0.1s of retrace + cache-deserialize).
    Inputs are still transferred and the NEFF executed on hardware each call.
    """
    n_cores = len(core_ids)
    if kw or list(core_ids) != list(range(n_cores)) or nc.dbg_addr is not None:
        return _lib_run_spmd(nc, in_maps, core_ids=core_ids, **kw)
    ent = _exec_cache.get(id(nc))
    if ent is None:
        from jax.sharding import Mesh, PartitionSpec
        from jax.experimental.shard_map import shard_map
        from concourse.bass2jax import (_bass_exec_p, install_neuronx_cc_hook,
                                        partition_id_tensor)
        install_neuronx_cc_hook()
        pname = (nc.partition_id_tensor.name if nc.partition_id_tensor
                 else None)
        in_names, out_names, out_avals, zero_outs = [], [], [], []
        for alloc in nc.m.functions[0].allocations:
            if not isinstance(alloc, mybir.MemoryLocationSet):
                continue
            name = alloc.memorylocations[0].name
            if alloc.kind == "ExternalInput":
                if name != pname:
                    in_names.append(name)
            elif alloc.kind == "ExternalOutput":
                out_names.append(name)
                shape = tuple(alloc.tensor_shape)
                dtype = mybir.dt.np(alloc.dtype)
                out_avals.append(jax.core.ShapedArray(shape, dtype))
                zero_outs.append(np.zeros(shape, dtype))
        n_params = len(in_names)
        all_names = in_names + out_names + ([pname] if pname else [])

        def _body(*args):
            operands = list(args)
            if pname is not None:
                operands.append(partition_id_tensor())
            outs = _bass_exec_p.bind(
                *operands, out_avals=tuple(out_avals),
                in_names=tuple(all_names), out_names=tuple(out_names),
                lowering_input_output_aliases=(), sim_require_finite=True,
                sim_require_nnan=True, nc=nc)
            return tuple(outs)

        devices = jax.devices()[:n_cores]
        mesh = Mesh(np.asarray(devices), ("core",))
        n_io = n_params + len(out_names)
        sharded = jax.jit(
            shard_map(_body, mesh=mesh,
                      in_specs=(PartitionSpec("core"),) * n_io,
                      out_specs=(PartitionSpec("core"),) * len(out_names),
                      check_rep=False),
            donate_argnums=tuple(range(n_params, n_io)), keep_unused=True)
        ent = (sharded, in_names, n_params, out_names, out_avals, zero_outs)
        _exec_cache[id(nc)] = ent
    sharded, in_names, n_params, out_names, out_avals, zero_outs = ent
    concat_in = [
        np.concatenate([np.asarray(in_maps[c][nm]) for c in range(n_cores)],
                       axis=0) for nm in in_names]
    concat_zeros = [np.zeros((n_cores * z.shape[0], *z.shape[1:]), z.dtype)
                    for z in zero_outs]
    out_arrs = sharded(*concat_in, *concat_zeros)
    results = [
        {nm: np.asarray(out_arrs[i]).reshape(n_cores, *out_avals[i].shape)[c]
         for i, nm in enumerate(out_names)}
        for c in range(n_cores)]
    return BassKernelResults(results=results, instructions_and_trace=None,
                             profile_json=None, exec_time_ns=None)


def _build():
    nc = bacc.Bacc("TRN2", target_bir_lowering=False, debug=False,
                   num_devices=NCORE)

    blob = nc.dram_tensor("blob", [BLOB_ROWS, 1024], BF,
                          kind="ExternalInput").ap()
    sheet = nc.dram_tensor("sheet", [SHEET_ROWS // NCORE, 1024], BF,
                           kind="ExternalInput").ap()
    blobS = nc.dram_tensor("blobS", [128, SCOLS], F32,
                           kind="ExternalInput").ap()
    out_shard = nc.dram_tensor("out_shard", [1, BN // NCORE], F32,
                               kind="ExternalOutput").ap()

    with tile.TileContext(nc) as tc:
        with (
            tc.tile_pool(name="dram", bufs=1, space="DRAM") as dram,
            tc.tile_pool(name="consts", bufs=1) as consts,
            tc.tile_pool(name="small", bufs=6) as small,
            tc.tile_pool(name="pp", bufs=4, space="PSUM") as pp,
            tc.tile_pool(name="pt", bufs=2, space="PSUM") as ppt,
        ):
            ident = consts.tile([128, 128], BF, name="ident", tag="ident")
            make_identity(nc, ident)
            pid = nc.sync.partition_id()

            # ---- gather the packed inputs across cores ----
            # (collectives cannot read IO tensors; bounce through DRAM scratch)
            blob_in = dram.tile([BLOB_ROWS, 1024], BF, name="blobi",
                                tag="blobi")
            nc.sync.dma_start(blob_in[:, :], blob[:, :])
            blob_out = dram.tile([NCORE * BLOB_ROWS, 1024], BF, name="blobo",
                                 tag="blobo", addr_space="Shared")
            nc.gpsimd.collective_compute(
                "AllGather", OP.bypass, replica_groups=ALL8,
                ins=[blob_in.opt()], outs=[blob_out.opt()])
            sheet_in = dram.tile([SHEET_ROWS // NCORE, 1024], BF,
                                 name="sheeti", tag="sheeti")
            nc.sync.dma_start(sheet_in[:, :], sheet[:, :])
            sheet_out = dram.tile([SHEET_ROWS, 1024], BF, name="sheeto",
                                  tag="sheeto", addr_space="Shared")
            nc.gpsimd.collective_compute(
                "AllGather", OP.bypass, replica_groups=ALL8,
                ins=[sheet_in.opt()], outs=[sheet_out.opt()])

            # clean (4096, 1024) m_cross scratch out of the gathered chunks
            mc_dram = dram.tile([N, 1024], BF, name="mcd", tag="mcd")
            for c in range(NCORE):
                nc.sync.dma_start(
                    mc_dram[c * 512:(c + 1) * 512, :],
                    blob_out[c * BLOB_ROWS:c * BLOB_ROWS + 512, :])

            blobS_sb = consts.tile([128, SCOLS], F32, name="bS", tag="bS")
            nc.sync.dma_start(blobS_sb[:], blobS[:, :])
            ncd_sb = consts.tile([128, 1], F32, name="ncd", tag="ncd")
            nc.sync.dma_start(ncd_sb[:], blobS[0:128, ds(SC_NCD + pid, 1)])
            ncu_sb = consts.tile([128, 1], F32, name="ncu", tag="ncu")
            nc.sync.dma_start(ncu_sb[:], blobS[0:128, ds(SC_NCU + pid, 1)])

            # thresholds broadcast to 128 partitions (ones ⊗ row via matmul)
            ones_sb = consts.tile([1, 128], BF, name="ones", tag="ones")
            nc.vector.memset(ones_sb[:], 1.0)
            td_row = consts.tile([1, 1024], BF, name="tdr", tag="tdr")
            nc.sync.dma_start(td_row[:], blob_out[R_TDOWN:R_TDOWN + 1, :])
            thrD = consts.tile([128, 1024], BF, name="thrD", tag="thrD")
            for hf in range(2):
                pb = pp.tile([128, 512], F32, name="pp", tag="pp")
                nc.tensor.matmul(pb[:], ones_sb[:],
                                 td_row[:, hf * 512:(hf + 1) * 512])
                nc.vector.tensor_copy(thrD[:, hf * 512:(hf + 1) * 512], pb[:])

            wen_sb = consts.tile([4, 256], BF, name="wen", tag="wen")
            nc.sync.dma_start(wen_sb[:], sheet_out[SH_MISC:SH_MISC + 4, 768:1024])
            wdn_sb = []
            wup_sb = []
            for t in range(2):
                w = consts.tile([128, KD], BF, name=f"wdn{t}", tag=f"wdn{t}")
                nc.sync.dma_start(
                    w[:], sheet_out[SH_MISC + t * 128:SH_MISC + (t + 1) * 128,
                                    ds(256 + pid * KD, KD)])
                wdn_sb.append(w)
                w = consts.tile([128, KD], BF, name=f"wupt{t}", tag=f"wupt{t}")
                nc.sync.dma_start(
                    w[:], sheet_out[SH_MISC + t * 128:SH_MISC + (t + 1) * 128,
                                    ds(512 + pid * KD, KD)])
                wup_sb.append(w)
            ben_sb = [blobS_sb[:, SC_BEN + t:SC_BEN + t + 1] for t in range(2)]

            TS = BN // NCORE  # 2048
            # ag1/ag4 use AllToAll (each core only needs its batch / token
            # slice), which shrinks the wire bytes 4-8x vs AllGather.
            ag1_in = dram.tile([NCORE * 32, M], BF, name="ag1i", tag="ag1i")
            ag1_out = dram.tile([NCORE * 32, M], BF, name="ag1o", tag="ag1o")
            ag3_in = dram.tile([D, M], BF, name="ag3i", tag="ag3i")
            ag3_out = dram.tile([NCORE * D, M], BF, name="ag3o", tag="ag3o",
                                addr_space="Shared")
            ag4_in = dram.tile([NCORE * KD, TS], BF, name="ag4i", tag="ag4i")
            ag4_out = dram.tile([NCORE * KD, TS], BF, name="ag4o", tag="ag4o")

            def psum(p, f, dt=F32):
                return pp.tile([p, f], dt, name="pp", tag="pp")

            _lwn = [0]

            def lw(pool, p0, p1, f0, f1, dt=BF):
                # load a (p1-p0, f1-f0) tile from sheet_out
                _lwn[0] += 1
                t = pool.tile([p1 - p0, f1 - f0], dt, name=f"lw{_lwn[0]}",
                              tag=f"lw{_lwn[0]}")
                nc.sync.dma_start(t[:], sheet_out[p0:p1, f0:f1])
                return t

            # ---------------- P1: down (head pid over full N) ----------------
            with tc.tile_pool(name="p1", bufs=3) as p1, \
                 tc.tile_pool(name="p1keep", bufs=1) as p1k:
                # encoder input (4, BN) from the 16 gathered groups
                enc_sb = p1k.tile([4, BN], BF, name="enc_sb", tag="enc_sb")
                for g in range(16):
                    src = (g // 2) * BLOB_ROWS + R_ENC + (g % 2) * 4
                    nc.sync.dma_start(enc_sb[:, g * 1024:(g + 1) * 1024],
                                      blob_out[src:src + 4, :])
                v_all = [p1k.tile([128, 4 * 33], BF, name=f"va{i}", tag=f"va{i}")
                         for i in range(32)]
                for b in range(B):
                    for ni in range(32):
                        off = b * N + ni * 128
                        enT = []
                        for t in range(2):
                            pe = psum(128, 128)
                            nc.tensor.matmul(
                                pe[:], wen_sb[:, t * 128:(t + 1) * 128],
                                enc_sb[:, off:off + 128])
                            g = p1.tile([128, 128], BF, name="enT", tag="enT")
                            nc.scalar.activation(g[:], pe[:], AF.Gelu,
                                                 bias=ben_sb[t])
                            enT.append(g)
                        pv = psum(128, KD)
                        for t in range(2):
                            nc.tensor.matmul(pv[:], enT[t][:], wdn_sb[t][:],
                                             start=(t == 0), stop=(t == 1))
                        nc.vector.tensor_copy(
                            v_all[ni][:, b * 33:b * 33 + KD], pv[:])
                        if b == 0:
                            for bb in range(B):
                                nc.vector.memset(
                                    v_all[ni][:, bb * 33 + 32:bb * 33 + 33],
                                    1.0)

                # a_sb[ni] = exp(-c_h * mc) * (mc <= thr)
                a_sb = [p1k.tile([128, M], BF, name=f"as{i}", tag=f"as{i}")
                        for i in range(32)]
                for ni in range(32):
                    mct = p1.tile([128, M], BF, name="mct", tag="mct")
                    nc.sync.dma_start(mct[:],
                                      mc_dram[ni * 128:(ni + 1) * 128, :])
                    msk = p1.tile([128, M], BF, name="msk", tag="msk")
                    nc.vector.tensor_tensor(msk[:], mct[:], thrD[:], OP.is_le)
                    nc.scalar.activation(a_sb[ni][:], mct[:], AF.Exp,
                                         scale=ncd_sb[:])
                    nc.vector.tensor_tensor(a_sb[ni][:], a_sb[ni][:], msk[:],
                                            OP.mult)

                xhT = p1k.tile([128, M], BF, name="xhT", tag="xhT")
                for mi in range(8):
                    px = psum(128, 4 * 33)
                    for ni in range(32):
                        nc.tensor.matmul(
                            px[:], a_sb[ni][:, mi * 128:(mi + 1) * 128],
                            v_all[ni][:], start=(ni == 0), stop=(ni == 31))
                    for b in range(B):
                        rc = small.tile([128, 1], F32, name="rc", tag="rc")
                        nc.vector.reciprocal(
                            rc[:], px[:, b * 33 + 32:b * 33 + 33])
                        gx = small.tile([128, KD], BF, name="gx", tag="gx")
                        nc.scalar.activation(gx[:], px[:, b * 33:b * 33 + KD],
                                             AF.Gelu, scale=rc[:])
                        ptr = ppt.tile([KD, 128], BF, name="tp", tag="tp")
                        nc.tensor.transpose(ptr[:], gx[:], ident[:])
                        nc.vector.tensor_copy(
                            xhT[b * 32:b * 32 + 32,
                                mi * 128:(mi + 1) * 128], ptr[:])
                # core h sends batch c//2's 32 feature rows to core c
                for c in range(NCORE):
                    nc.sync.dma_start(
                        ag1_in[c * 32:(c + 1) * 32, :],
                        xhT[(c // 2) * 32:(c // 2) * 32 + 32, :])
                nc.gpsimd.collective_compute(
                    "AllToAll", OP.bypass, replica_groups=ALL8,
                    ins=[ag1_in.opt()], outs=[ag1_out.opt()])

            # ---------------- P2: blocks ----------------
            b0x32 = (pid // 2) * 32
            # static range clamp only — the emitted runtime-assert sequencer
            # instruction faults the exec unit on this runtime, so skip it
            hcol = nc.s_assert_within((pid - (pid // 2) * 2) * 128, 0, 128,
                                      skip_runtime_assert=True)
            with tc.tile_pool(name="p2", bufs=2) as p2, \
                 tc.tile_pool(name="p2e", bufs=1) as p2e:
                xT = [p2e.tile([128, M], BF, name=f"xT{t}", tag=f"xT{t}")
                      for t in range(2)]
                for hh in range(H):
                    nc.sync.dma_start(
                        xT[hh // 4][(hh % 4) * 32:(hh % 4) * 32 + 32, :],
                        ag1_out[hh * 32:(hh + 1) * 32, :])

                for blk in range(NB):
                    qp_sb, kp_sb, vp_sb = [], [], []
                    for t in range(2):
                        for dst, base in ((qp_sb, SH_QP), (kp_sb, SH_KP),
                                          (vp_sb, SH_VP)):
                            _lwn[0] += 1
                            w = p2.tile([128, 128], BF, name=f"lw{_lwn[0]}",
                                        tag=f"lw{_lwn[0]}")
                            nc.sync.dma_start(
                                w[:], sheet_out[base + t * 128:
                                                base + (t + 1) * 128,
                                                ds(blk * 256 + hcol, 128)])
                            dst.append(w)

                    qt = p2e.tile([128, M], BF, name="qt", tag="qt")
                    kt = p2e.tile([128, M], BF, name="kt", tag="kt")
                    for dst, wsb in ((qt, qp_sb), (kt, kp_sb)):
                        for mh in range(2):
                            pq = psum(128, 512)
                            for t in range(2):
                                nc.tensor.matmul(
                                    pq[:], wsb[t][:],
                                    xT[t][:, mh * 512:(mh + 1) * 512],
                                    start=(t == 0), stop=(t == 1))
                            nc.vector.tensor_copy(
                                dst[:, mh * 512:(mh + 1) * 512], pq[:])
                    qh = [p2e.tile([KD, M], BF, name=f"qh{h}", tag=f"qh{h}")
                          for h in range(4)]
                    kh = [p2e.tile([KD, M], BF, name=f"kh{h}", tag=f"kh{h}")
                          for h in range(4)]
                    for h in range(4):
                        nc.vector.tensor_copy(qh[h][:],
                                              qt[h * 32:h * 32 + 32, :])
                        nc.vector.tensor_copy(kh[h][:],
                                              kt[h * 32:h * 32 + 32, :])

                    vh = [p2e.tile([128, 4 * 33], BF, name=f"vh{ni}",
                                   tag=f"vh{ni}") for ni in range(8)]
                    for ni in range(8):
                        pvv = psum(128, 128)
                        for t in range(2):
                            nc.tensor.matmul(
                                pvv[:],
                                xT[t][:, ni * 128:(ni + 1) * 128],
                                vp_sb[t][:], start=(t == 0), stop=(t == 1))
                        for h in range(4):
                            nc.vector.tensor_copy(
                                vh[ni][:, h * 33:h * 33 + KD],
                                pvv[:, h * 32:h * 32 + 32])
                            nc.vector.memset(
                                vh[ni][:, h * 33 + 32:h * 33 + 33], 1.0)

                    es = [[p2e.tile([128, M], BF, name=f"es{h}_{ni}",
                                    tag=f"es{h}_{ni}")
                           for ni in range(8)] for h in range(4)]
                    for h in range(4):
                        for ni in range(8):
                            for mh in range(2):
                                psc = psum(128, 512)
                                nc.tensor.matmul(
                                    psc[:],
                                    kh[h][:, ni * 128:(ni + 1) * 128],
                                    qh[h][:, mh * 512:(mh + 1) * 512])
                                nc.scalar.activation(
                                    es[h][ni][:, mh * 512:(mh + 1) * 512],
                                    psc[:], AF.Exp, scale=INV_SQRT_K)

                    paT = p2e.tile([128, M], BF, name="paT", tag="paT")
                    for mi in range(8):
                        for h in range(4):
                            pa = psum(128, 33)
                            for ni in range(8):
                                nc.tensor.matmul(
                                    pa[:],
                                    es[h][ni][:, mi * 128:(mi + 1) * 128],
                                    vh[ni][:, h * 33:h * 33 + 33],
                                    start=(ni == 0), stop=(ni == 7))
                            rc = small.tile([128, 1], F32, name="rc", tag="rc")
                            nc.vector.reciprocal(rc[:], pa[:, 32:33])
                            gx = small.tile([128, KD], BF, name="gx", tag="gx")
                            nc.scalar.activation(gx[:], pa[:, 0:KD], AF.Gelu,
                                                 scale=rc[:])
                            ptr = ppt.tile([KD, 128], BF, name="tp", tag="tp")
                            nc.tensor.transpose(ptr[:], gx[:], ident[:])
                            nc.vector.tensor_copy(
                                paT[h * 32:h * 32 + 32,
                                    mi * 128:(mi + 1) * 128], ptr[:])

                    ag2_in = dram.tile([128, M], BF, name="ag2i", tag="ag2i")
                    ag2_out = dram.tile([D, M], BF, name="ag2o", tag="ag2o")
                    nc.sync.dma_start(ag2_in[:], paT[:])
                    nc.gpsimd.collective_compute(
                        "AllGather", OP.bypass,
                        replica_groups=PAIRS,
                        ins=[ag2_in.opt()], outs=[ag2_out.opt()])
                    paF = [p2e.tile([128, M], BF, name=f"paF{t}", tag=f"paF{t}")
                           for t in range(2)]
                    for t in range(2):
                        nc.sync.dma_start(
                            paF[t][:], ag2_out[t * 128:(t + 1) * 128, :])

                    w1_sb = [[lw(p2, SH_W1 + i * 128, SH_W1 + (i + 1) * 128,
                                 blk * 256 + o * 128, blk * 256 + (o + 1) * 128)
                              for o in range(2)] for i in range(2)]
                    b1_sb = [blobS_sb[:, SC_B1 + blk * 2 + t:
                                      SC_B1 + blk * 2 + t + 1]
                             for t in range(2)]
                    h1 = [p2e.tile([128, M], BF, name=f"h1{t}", tag=f"h1{t}")
                          for t in range(2)]
                    for o in range(2):
                        for mh in range(2):
                            ph = psum(128, 512)
                            for i in range(2):
                                nc.tensor.matmul(
                                    ph[:], w1_sb[i][o][:],
                                    paF[i][:, mh * 512:(mh + 1) * 512],
                                    start=(i == 0), stop=(i == 1))
                            nc.scalar.activation(
                                h1[o][:, mh * 512:(mh + 1) * 512], ph[:],
                                AF.Gelu, bias=b1_sb[o])

                    w2_sb = [[lw(p2, SH_W2 + i * 128, SH_W2 + (i + 1) * 128,
                                 blk * 256 + o * 128, blk * 256 + (o + 1) * 128)
                              for o in range(2)] for i in range(2)]
                    wr_sb = [[lw(p2, SH_WR + i * 128, SH_WR + (i + 1) * 128,
                                 blk * 256 + o * 128, blk * 256 + (o + 1) * 128)
                              for o in range(2)] for i in range(2)]
                    bc_sb = [blobS_sb[:, SC_BC + blk * 2 + t:
                                      SC_BC + blk * 2 + t + 1]
                             for t in range(2)]
                    xn = [p2e.tile([128, M], BF, name=f"xn{t}", tag=f"xn{t}")
                          for t in range(2)]
                    for o in range(2):
                        for mh in range(2):
                            po = psum(128, 512)
                            nc.tensor.matmul(
                                po[:], w2_sb[0][o][:],
                                h1[0][:, mh * 512:(mh + 1) * 512],
                                start=True, stop=False)
                            nc.tensor.matmul(
                                po[:], w2_sb[1][o][:],
                                h1[1][:, mh * 512:(mh + 1) * 512],
                                start=False, stop=False)
                            nc.tensor.matmul(
                                po[:], wr_sb[0][o][:],
                                xT[0][:, mh * 512:(mh + 1) * 512],
                                start=False, stop=False)
                            nc.tensor.matmul(
                                po[:], wr_sb[1][o][:],
                                xT[1][:, mh * 512:(mh + 1) * 512],
                                start=False, stop=True)
                            nc.scalar.activation(
                                xn[o][:, mh * 512:(mh + 1) * 512], po[:],
                                AF.Gelu, bias=bc_sb[o])
                    xT = xn

                for t in range(2):
                    nc.sync.dma_start(ag3_in[t * 128:(t + 1) * 128, :],
                                      xT[t][:])
                nc.gpsimd.collective_compute(
                    "AllGather", OP.bypass, replica_groups=ALL8,
                    ins=[ag3_in.opt()], outs=[ag3_out.opt()])

            # ---------------- P4: up (head pid over full N) ----------------
            with tc.tile_pool(name="p4", bufs=3) as p4, \
                 tc.tile_pool(name="p4keep", bufs=1) as p4k:
                vu = [p4k.tile([128, 4 * 33], BF, name=f"vu{i}", tag=f"vu{i}")
                      for i in range(8)]
                for b in range(B):
                    xb = [p4.tile([128, M], BF, name=f"xb{t}", tag=f"xb{t}")
                          for t in range(2)]
                    for t in range(2):
                        nc.sync.dma_start(
                            xb[t][:],
                            ag3_out[2 * b * D + t * 128:
                                    2 * b * D + (t + 1) * 128, :])
                    for mi in range(8):
                        pv = psum(128, KD)
                        for t in range(2):
                            nc.tensor.matmul(
                                pv[:], xb[t][:, mi * 128:(mi + 1) * 128],
                                wup_sb[t][:], start=(t == 0), stop=(t == 1))
                        nc.vector.tensor_copy(
                            vu[mi][:, b * 33:b * 33 + KD], pv[:])
                        if b == 0:
                            for bb in range(B):
                                nc.vector.memset(
                                    vu[mi][:, bb * 33 + 32:bb * 33 + 33], 1.0)

                # thrU (128, N) from the gathered t_up rows (matmul broadcast)
                tu_row = p4k.tile([1, N], BF, name="tur", tag="tur")
                for r in range(4):
                    nc.sync.dma_start(
                        tu_row[:, r * 1024:(r + 1) * 1024],
                        blob_out[R_TUP + r:R_TUP + r + 1, :])
                thrU = p4k.tile([128, N], BF, name="thrU", tag="thrU")
                for hf in range(8):
                    pb = pp.tile([128, 512], F32, name="pp", tag="pp")
                    nc.tensor.matmul(pb[:], ones_sb[:],
                                     tu_row[:, hf * 512:(hf + 1) * 512])
                    nc.vector.tensor_copy(thrU[:, hf * 512:(hf + 1) * 512],
                                          pb[:])

                # eu[ki] = exp(-c_h * mcT) * (mcT <= thrU), mcT via DMA transpose
                eu = [p4k.tile([128, N], BF, name=f"eu{i}", tag=f"eu{i}")
                      for i in range(8)]
                for ki in range(8):
                    mctT = p4.tile([128, N], BF, name="mctT", tag="mctT")
                    nc.sync.dma_start(mctT[:],
                                      mc_dram[:, ki * 128:(ki + 1) * 128],
                                      transpose=True)
                    msk = p4.tile([128, N], BF, name="umsk", tag="umsk")
                    nc.vector.tensor_tensor(msk[:], mctT[:], thrU[:], OP.is_le)
                    nc.scalar.activation(eu[ki][:], mctT[:], AF.Exp,
                                         scale=ncu_sb[:])
                    nc.vector.tensor_tensor(eu[ki][:], eu[ki][:], msk[:],
                                            OP.mult)

                deT = p4k.tile([KD, BN], BF, name="deT", tag="deT")
                for qi in range(32):
                    pd = psum(128, 4 * 33)
                    for ki in range(8):
                        nc.tensor.matmul(
                            pd[:], eu[ki][:, qi * 128:(qi + 1) * 128],
                            vu[ki][:], start=(ki == 0), stop=(ki == 7))
                    for b in range(B):
                        rc = small.tile([128, 1], F32, name="rc", tag="rc")
                        nc.vector.reciprocal(
                            rc[:], pd[:, b * 33 + 32:b * 33 + 33])
                        gx = small.tile([128, KD], BF, name="gx", tag="gx")
                        nc.scalar.activation(gx[:], pd[:, b * 33:b * 33 + KD],
                                             AF.Gelu, scale=rc[:])
                        ptr = ppt.tile([KD, 128], BF, name="tp", tag="tp")
                        nc.tensor.transpose(ptr[:], gx[:], ident[:])
                        nc.vector.tensor_copy(
                            deT[:, b * N + qi * 128:b * N + (qi + 1) * 128],
                            ptr[:])
                # core h sends token chunk c (its 32 de^T rows) to core c
                for c in range(NCORE):
                    nc.sync.dma_start(ag4_in[c * 32:(c + 1) * 32, :],
                                      deT[:, c * TS:(c + 1) * TS])
                nc.gpsimd.collective_compute(
                    "AllToAll", OP.bypass, replica_groups=ALL8,
                    ins=[ag4_in.opt()], outs=[ag4_out.opt()])

            # ---------------- P5: decoder on token shard ----------------
            with tc.tile_pool(name="p5", bufs=2) as p5:
                dea = [p5.tile([128, TS], BF, name=f"dea{t}", tag=f"dea{t}")
                       for t in range(2)]
                for t in range(2):
                    nc.sync.dma_start(
                        dea[t][:],
                        ag4_out[t * 128:(t + 1) * 128, :])
                wd1 = [[lw(p5, SH_MISC + i * 128, SH_MISC + (i + 1) * 128,
                           o * 128, (o + 1) * 128) for o in range(2)]
                       for i in range(2)]
                bd1 = [blobS_sb[:, SC_BD1 + t:SC_BD1 + t + 1]
                       for t in range(2)]
                wd2 = [lw(p5, SH_MISC + 8, SH_MISC + 8 + 128,
                          770 + t, 771 + t) for t in range(2)]
                g = [p5.tile([128, TS], BF, name=f"g{t}", tag=f"g{t}")
                     for t in range(2)]
                for o in range(2):
                    for th in range(4):
                        pg = psum(128, 512)
                        for i in range(2):
                            nc.tensor.matmul(
                                pg[:], wd1[i][o][:],
                                dea[i][:, th * 512:(th + 1) * 512],
                                start=(i == 0), stop=(i == 1))
                        nc.scalar.activation(
                            g[o][:, th * 512:(th + 1) * 512], pg[:],
                            AF.Gelu, bias=bd1[o])
                osb = p5.tile([1, TS], F32, name="osb", tag="osb")
                for th in range(4):
                    p2o = psum(1, 512)
                    for i in range(2):
                        nc.tensor.matmul(
                            p2o[:], wd2[i][:],
                            g[i][:, th * 512:(th + 1) * 512],
                            start=(i == 0), stop=(i == 1))
                    nc.vector.tensor_copy(
                        osb[:, th * 512:(th + 1) * 512], p2o[:])
                nc.sync.dma_start(out_shard[:, :], osb[:])

    nc.compile()
    return nc


def _prep_inputs(inputs, m_cross, W_en, b_en, r_down, w_down, q_pa, k_pa,
                 v_pa, W1_mlp, b1_mlp, W2_mlp, b2_mlp, W_res, b_res, r_up,
                 w_up, W_de1, b_de1, W_de2, b_de2, y_mean, y_std):
    f32 = np.float32
    mc = np.asarray(m_cross, f32)
    mcb = mc.astype(NPBF)
    mcbf = mcb.astype(f32)

    # bf16-robust percentile thresholds: max passing bf16 value so the
    # device-side (bf16 mc <= thr) comparison reproduces the f32 mask
    kd_ = int(0.30 * (N - 1))          # 1228
    vkd = np.partition(mc, kd_, axis=0)[kd_, :]               # (M,)
    t_down_dev = np.where(mc <= vkd[None, :], mcbf, -np.inf).max(axis=0)
    ku_ = int(0.30 * (M - 1))          # 306
    vku = np.partition(mc, ku_, axis=1)[:, ku_]               # (N,)
    t_up_dev = np.where(mc <= vku[:, None], mcbf, -np.inf).max(axis=1)

    # encoder input (4, BN), group-major rows of 1024
    gx = np.linspace(0.0, 1.0, RES + 1, dtype=f32)[:-1]
    gxx = np.broadcast_to(gx[:, None], (RES, RES))
    gyy = np.broadcast_to(gx[None, :], (RES, RES))
    encf = np.zeros((4, BN), f32)
    encf[0, :] = np.tile(gxx.reshape(-1), B)
    encf[1, :] = np.tile(gyy.reshape(-1), B)
    encf[2, :] = np.asarray(inputs, f32).reshape(BN)
    enc_cat = encf.reshape(4, 16, 1024).transpose(1, 0, 2).reshape(64, 1024)

    c_down = np.tan(0.25 * np.pi * (1.0 + np.sin(np.asarray(r_down, f32)
                                                 .reshape(H)))).astype(f32)
    c_up = np.tan(0.25 * np.pi * (1.0 + np.sin(np.asarray(r_up, f32)
                                               .reshape(H)))).astype(f32)

    # ---- weight sheet (1792, 1024) ----
    sheet = np.zeros((SHEET_ROWS, 1024), NPBF)

    def cat_blocks(w):  # (NB, D, D) -> (D, NB*D)
        return np.asarray(w, f32).transpose(1, 0, 2).reshape(D, NB * D)

    def cat_heads(w):   # (NB, H, D, K) -> (D, NB*H*K)
        return np.asarray(w, f32).transpose(2, 0, 1, 3).reshape(D, NB * H * KD)

    sheet[SH_W1:SH_W1 + D] = cat_blocks(W1_mlp).astype(NPBF)
    sheet[SH_W2:SH_W2 + D] = cat_blocks(W2_mlp).astype(NPBF)
    sheet[SH_WR:SH_WR + D] = cat_blocks(W_res).astype(NPBF)
    sheet[SH_QP:SH_QP + D] = cat_heads(q_pa).astype(NPBF)
    sheet[SH_KP:SH_KP + D] = cat_heads(k_pa).astype(NPBF)
    sheet[SH_VP:SH_VP + D] = cat_heads(v_pa).astype(NPBF)
    sheet[SH_MISC:SH_MISC + D, 0:256] = np.asarray(W_de1, f32).astype(NPBF)
    sheet[SH_MISC:SH_MISC + D, 256:512] = (
        np.asarray(w_down, f32).transpose(1, 0, 2).reshape(D, H * KD)
        .astype(NPBF))
    sheet[SH_MISC:SH_MISC + D, 512:768] = (
        np.asarray(w_up, f32).transpose(1, 0, 2).reshape(D, H * KD)
        .astype(NPBF))
    wen4 = np.zeros((4, D), f32)
    wen4[:3, :] = np.asarray(W_en, f32)
    sheet[SH_MISC:SH_MISC + 4, 768:1024] = wen4.astype(NPBF)
    ystd = float(np.asarray(y_std, f32))
    ymean = float(np.asarray(y_mean, f32))
    wde2f = (np.asarray(W_de2, f32).reshape(D) * ystd).astype(NPBF)
    sheet[SH_MISC + 8:SH_MISC + 8 + 128, 770] = wde2f[0:128]
    sheet[SH_MISC + 8:SH_MISC + 8 + 128, 771] = wde2f[128:256]
    bde2f = float(np.asarray(b_de2, f32).reshape(-1)[0] * ystd + ymean)

    # ---- f32 constants (128, 36) ----
    bS = np.zeros((128, SCOLS), f32)
    bS[:, SC_NCD:SC_NCD + 8] = -c_down[None, :]
    bS[:, SC_NCU:SC_NCU + 8] = -c_up[None, :]
    ben = np.asarray(b_en, f32).reshape(D)
    bS[:, SC_BEN] = ben[0:128]
    bS[:, SC_BEN + 1] = ben[128:256]
    b1f = np.asarray(b1_mlp, f32).reshape(NB, D)
    bcf = (np.asarray(b2_mlp, f32) + np.asarray(b_res, f32)).reshape(NB, D)
    for blk in range(NB):
        for t in range(2):
            bS[:, SC_B1 + blk * 2 + t] = b1f[blk, t * 128:(t + 1) * 128]
            bS[:, SC_BC + blk * 2 + t] = bcf[blk, t * 128:(t + 1) * 128]
    bd1 = np.asarray(b_de1, f32).reshape(D)
    bS[:, SC_BD1] = bd1[0:128]
    bS[:, SC_BD1 + 1] = bd1[128:256]

    in_maps = []
    for c in range(NCORE):
        blob = np.zeros((BLOB_ROWS, 1024), NPBF)
        blob[0:512] = mcb[c * 512:(c + 1) * 512]
        blob[R_ENC:R_ENC + 8] = enc_cat[c * 8:(c + 1) * 8].astype(NPBF)
        blob[R_TDOWN] = t_down_dev.astype(NPBF)
        blob[R_TUP:R_TUP + 4] = t_up_dev.reshape(4, 1024).astype(NPBF)
        in_maps.append({
            "blob": blob,
            "sheet": sheet[c * (SHEET_ROWS // NCORE):
                           (c + 1) * (SHEET_ROWS // NCORE)],
            "blobS": bS,
        })
    return in_maps, bde2f


def kernel(**inputs):
    if "nc" not in _cache:
        _cache["nc"] = _build()
    nc = _cache["nc"]
    in_maps, bde2f = _prep_inputs(**inputs)
    res = run_bass_kernel_spmd(nc, in_maps, core_ids=list(range(NCORE)))
    shards = [res.results[c]["out_shard"].reshape(-1) + np.float32(bde2f)
              for c in range(NCORE)]
    out = np.concatenate(shards).astype(np.float32)
    return out.reshape(B, RES, RES, 1)


# revision 17
# speedup vs baseline: 1.6803x; 1.0038x over previous
"""Trainium2 Bass kernel for nn_LiteTransformer (sparse_attention).

Sharding (8 cores):
  - position-attention (down & up): by head (core c owns head c)
  - self-attention blocks: core c owns batch c//2, heads c%2*4..+4
  - decoder: token-sharded (2048 tokens per core)

Host->device transfer is the wall-clock bottleneck (axon tunnel ~120MB/s,
~0.2s per tensor latency), so inputs are packed into THREE small tensors
per core (~1.5MB total) instead of replicating the 16MB masked distance
matrices everywhere:
  - blob  (525,1024) bf16: m_cross row-shard + encoder-input slice +
    masked-percentile thresholds; AllGathered on device.
  - sheet (224,1024) bf16: 1/8 shard of all weights; AllGathered on device.
  - blobS (128,36)  f32 : per-head exp scales + biases (replicated).
The masked distance matrices exp(-c*(mc + big*(mc>thr))) are computed
on-device as exp(-c*mc) * (mc <= thr); thresholds are host-refined so the
bf16 comparison reproduces the exact f32 percentile mask.
"""

import numpy as np
import ml_dtypes

import jax
# run_bass_kernel_spmd builds a fresh jit closure per call; persist the XLA
# executable so repeat calls skip the ~0.5s re-compile (NEFF is already
# disk-cached separately).
jax.config.update("jax_compilation_cache_dir", "/tmp/jax_cache_kernel")
jax.config.update("jax_persistent_cache_min_entry_size_bytes", 0)
jax.config.update("jax_persistent_cache_min_compile_time_secs", 0)

import concourse.bass as bass
import concourse.mybir as mybir
import concourse.tile as tile
from concourse import bacc
from concourse.bass import ds
from concourse.bass_utils import (run_bass_kernel_spmd as _lib_run_spmd,
                                  BassKernelResults)
from concourse.masks import make_identity

BF = mybir.dt.bfloat16
F32 = mybir.dt.float32
AF = mybir.ActivationFunctionType
OP = mybir.AluOpType
NPBF = ml_dtypes.bfloat16

B, RES, N, M, H, D, KD, NB = 4, 64, 4096, 1024, 8, 256, 32, 4
BN = B * N
NCORE = 8
INV_SQRT_K = float(1.0 / np.sqrt(np.float32(KD)))
ALL8 = [list(range(NCORE))]
PAIRS = [[0, 1], [2, 3], [4, 5], [6, 7]]

# blob layout (per-core rows, width 1024 bf16)
BLOB_ROWS = 525          # 512 mc + 8 enc + 1 t_down + 4 t_up
R_ENC = 512
R_TDOWN = 520
R_TUP = 521
# sheet layout (global rows, width 1024 bf16)
SHEET_ROWS = 1792        # 224 per core
SH_W1, SH_W2, SH_WR = 0, 256, 512
SH_QP, SH_KP, SH_VP = 768, 1024, 1280
SH_MISC = 1536           # cols 0:256 wde1 | 256:512 wdown | 512:768 wup | 768: misc2
# misc2: rows SH_MISC..+4 cols 768:1024 = wen; wde2 halves at cols 770,771 rows +8..+136
# blobS cols
SC_NCD, SC_NCU, SC_BEN, SC_B1, SC_BC, SC_BD1, SCOLS = 0, 8, 16, 18, 26, 34, 36

_cache = {}
_exec_cache = {}


def run_bass_kernel_spmd(nc, in_maps, core_ids, **kw):
    """Same semantics as bass_utils.run_bass_kernel_spmd for the plain SPMD
    case, but keeps the jitted executable across calls (the library builds a
    fresh closure per call, costing ~0.1s of retrace + cache-deserialize).
    Inputs are still transferred and the NEFF executed on hardware each call.
    """
    n_cores = len(core_ids)
    if kw or list(core_ids) != list(range(n_cores)) or nc.dbg_addr is not None:
        return _lib_run_spmd(nc, in_maps, core_ids=core_ids, **kw)
    ent = _exec_cache.get(id(nc))
    if ent is None:
        from jax.sharding import Mesh, PartitionSpec
        from jax.experimental.shard_map import shard_map
        from concourse.bass2jax import (_bass_exec_p, install_neuronx_cc_hook,
                                        partition_id_tensor)
        install_neuronx_cc_hook()
        pname = (nc.partition_id_tensor.name if nc.partition_id_tensor
                 else None)
        in_names, out_names, out_avals, zero_outs = [], [], [], []
        for alloc in nc.m.functions[0].allocations:
            if not isinstance(alloc, mybir.MemoryLocationSet):
                continue
            name = alloc.memorylocations[0].name
            if alloc.kind == "ExternalInput":
                if name != pname:
                    in_names.append(name)
            elif alloc.kind == "ExternalOutput":
                out_names.append(name)
                shape = tuple(alloc.tensor_shape)
                dtype = mybir.dt.np(alloc.dtype)
                out_avals.append(jax.core.ShapedArray(shape, dtype))
                zero_outs.append(np.zeros(shape, dtype))
        n_params = len(in_names)
        all_names = in_names + out_names + ([pname] if pname else [])

        def _body(*args):
            operands = list(args)
            if pname is not None:
                operands.append(partition_id_tensor())
            outs = _bass_exec_p.bind(
                *operands, out_avals=tuple(out_avals),
                in_names=tuple(all_names), out_names=tuple(out_names),
                lowering_input_output_aliases=(), sim_require_finite=True,
                sim_require_nnan=True, nc=nc)
            return tuple(outs)

        devices = jax.devices()[:n_cores]
        mesh = Mesh(np.asarray(devices), ("core",))
        n_io = n_params + len(out_names)
        sharded = jax.jit(
            shard_map(_body, mesh=mesh,
                      in_specs=(PartitionSpec("core"),) * n_io,
                      out_specs=(PartitionSpec("core"),) * len(out_names),
                      check_rep=False),
            donate_argnums=tuple(range(n_params, n_io)), keep_unused=True)
        ent = (sharded, in_names, n_params, out_names, out_avals, zero_outs)
        _exec_cache[id(nc)] = ent
    sharded, in_names, n_params, out_names, out_avals, zero_outs = ent
    concat_in = [
        np.concatenate([np.asarray(in_maps[c][nm]) for c in range(n_cores)],
                       axis=0) for nm in in_names]
    concat_zeros = [np.zeros((n_cores * z.shape[0], *z.shape[1:]), z.dtype)
                    for z in zero_outs]
    out_arrs = sharded(*concat_in, *concat_zeros)
    results = [
        {nm: np.asarray(out_arrs[i]).reshape(n_cores, *out_avals[i].shape)[c]
         for i, nm in enumerate(out_names)}
        for c in range(n_cores)]
    return BassKernelResults(results=results, instructions_and_trace=None,
                             profile_json=None, exec_time_ns=None)


def _build():
    nc = bacc.Bacc("TRN2", target_bir_lowering=False, debug=False,
                   num_devices=NCORE)

    blob = nc.dram_tensor("blob", [BLOB_ROWS, 1024], BF,
                          kind="ExternalInput").ap()
    sheet = nc.dram_tensor("sheet", [SHEET_ROWS // NCORE, 1024], BF,
                           kind="ExternalInput").ap()
    blobS = nc.dram_tensor("blobS", [128, SCOLS], F32,
                           kind="ExternalInput").ap()
    out_shard = nc.dram_tensor("out_shard", [1, BN // NCORE], F32,
                               kind="ExternalOutput").ap()

    with tile.TileContext(nc) as tc:
        with (
            tc.tile_pool(name="dram", bufs=1, space="DRAM") as dram,
            tc.tile_pool(name="consts", bufs=1) as consts,
            tc.tile_pool(name="small", bufs=6) as small,
            tc.tile_pool(name="pp", bufs=4, space="PSUM") as pp,
            tc.tile_pool(name="pt", bufs=2, space="PSUM") as ppt,
        ):
            ident = consts.tile([128, 128], BF, name="ident", tag="ident")
            make_identity(nc, ident)
            pid = nc.sync.partition_id()

            # ---- gather the packed inputs across cores ----
            # (collectives cannot read IO tensors; bounce through DRAM scratch)
            blob_in = dram.tile([BLOB_ROWS, 1024], BF, name="blobi",
                                tag="blobi")
            nc.sync.dma_start(blob_in[:, :], blob[:, :])
            blob_out = dram.tile([NCORE * BLOB_ROWS, 1024], BF, name="blobo",
                                 tag="blobo", addr_space="Shared")
            nc.gpsimd.collective_compute(
                "AllGather", OP.bypass, replica_groups=ALL8,
                ins=[blob_in.opt()], outs=[blob_out.opt()])
            sheet_in = dram.tile([SHEET_ROWS // NCORE, 1024], BF,
                                 name="sheeti", tag="sheeti")
            nc.sync.dma_start(sheet_in[:, :], sheet[:, :])
            sheet_out = dram.tile([SHEET_ROWS, 1024], BF, name="sheeto",
                                  tag="sheeto", addr_space="Shared")
            nc.gpsimd.collective_compute(
                "AllGather", OP.bypass, replica_groups=ALL8,
                ins=[sheet_in.opt()], outs=[sheet_out.opt()])

            # clean (4096, 1024) m_cross scratch out of the gathered chunks
            mc_dram = dram.tile([N, 1024], BF, name="mcd", tag="mcd")
            for c in range(NCORE):
                nc.sync.dma_start(
                    mc_dram[c * 512:(c + 1) * 512, :],
                    blob_out[c * BLOB_ROWS:c * BLOB_ROWS + 512, :])

            blobS_sb = consts.tile([128, SCOLS], F32, name="bS", tag="bS")
            nc.sync.dma_start(blobS_sb[:], blobS[:, :])
            ncd_sb = consts.tile([128, 1], F32, name="ncd", tag="ncd")
            nc.sync.dma_start(ncd_sb[:], blobS[0:128, ds(SC_NCD + pid, 1)])
            ncu_sb = consts.tile([128, 1], F32, name="ncu", tag="ncu")
            nc.sync.dma_start(ncu_sb[:], blobS[0:128, ds(SC_NCU + pid, 1)])

            # thresholds broadcast to 128 partitions (ones ⊗ row via matmul)
            ones_sb = consts.tile([1, 128], BF, name="ones", tag="ones")
            nc.vector.memset(ones_sb[:], 1.0)
            td_row = consts.tile([1, 1024], BF, name="tdr", tag="tdr")
            nc.sync.dma_start(td_row[:], blob_out[R_TDOWN:R_TDOWN + 1, :])
            thrD = consts.tile([128, 1024], BF, name="thrD", tag="thrD")
            for hf in range(2):
                pb = pp.tile([128, 512], F32, name="pp", tag="pp")
                nc.tensor.matmul(pb[:], ones_sb[:],
                                 td_row[:, hf * 512:(hf + 1) * 512])
                nc.vector.tensor_copy(thrD[:, hf * 512:(hf + 1) * 512], pb[:])

            wen_sb = consts.tile([4, 256], BF, name="wen", tag="wen")
            nc.sync.dma_start(wen_sb[:], sheet_out[SH_MISC:SH_MISC + 4, 768:1024])
            wdn_sb = []
            wup_sb = []
            for t in range(2):
                w = consts.tile([128, KD], BF, name=f"wdn{t}", tag=f"wdn{t}")
                nc.sync.dma_start(
                    w[:], sheet_out[SH_MISC + t * 128:SH_MISC + (t + 1) * 128,
                                    ds(256 + pid * KD, KD)])
                wdn_sb.append(w)
                w = consts.tile([128, KD], BF, name=f"wupt{t}", tag=f"wupt{t}")
                nc.sync.dma_start(
                    w[:], sheet_out[SH_MISC + t * 128:SH_MISC + (t + 1) * 128,
                                    ds(512 + pid * KD, KD)])
                wup_sb.append(w)
            ben_sb = [blobS_sb[:, SC_BEN + t:SC_BEN + t + 1] for t in range(2)]

            TS = BN // NCORE  # 2048
            # ag1/ag4 use AllToAll (each core only needs its batch / token
            # slice), which shrinks the wire bytes 4-8x vs AllGather.
            ag1_in = dram.tile([NCORE * 32, M], BF, name="ag1i", tag="ag1i")
            ag1_out = dram.tile([NCORE * 32, M], BF, name="ag1o", tag="ag1o")
            ag3_in = dram.tile([D, M], BF, name="ag3i", tag="ag3i")
            ag3_out = dram.tile([NCORE * D, M], BF, name="ag3o", tag="ag3o",
                                addr_space="Shared")
            ag4_in = dram.tile([NCORE * KD, TS], BF, name="ag4i", tag="ag4i")
            ag4_out = dram.tile([NCORE * KD, TS], BF, name="ag4o", tag="ag4o")

            def psum(p, f, dt=F32):
                return pp.tile([p, f], dt, name="pp", tag="pp")

            _lwn = [0]

            def lw(pool, p0, p1, f0, f1, dt=BF):
                # load a (p1-p0, f1-f0) tile from sheet_out
                _lwn[0] += 1
                t = pool.tile([p1 - p0, f1 - f0], dt, name=f"lw{_lwn[0]}",
                              tag=f"lw{_lwn[0]}")
                nc.sync.dma_start(t[:], sheet_out[p0:p1, f0:f1])
                return t

            # ---------------- P1: down (head pid over full N) ----------------
            with tc.tile_pool(name="p1", bufs=3) as p1, \
                 tc.tile_pool(name="p1keep", bufs=1) as p1k:
                # encoder input (4, BN) from the 16 gathered groups
                enc_sb = p1k.tile([4, BN], BF, name="enc_sb", tag="enc_sb")
                for g in range(16):
                    src = (g // 2) * BLOB_ROWS + R_ENC + (g % 2) * 4
                    nc.sync.dma_start(enc_sb[:, g * 1024:(g + 1) * 1024],
                                      blob_out[src:src + 4, :])
                v_all = [p1k.tile([128, 4 * 33], BF, name=f"va{i}", tag=f"va{i}")
                         for i in range(32)]
                for b in range(B):
                    for ni in range(32):
                        off = b * N + ni * 128
                        enT = []
                        for t in range(2):
                            pe = psum(128, 128)
                            nc.tensor.matmul(
                                pe[:], wen_sb[:, t * 128:(t + 1) * 128],
                                enc_sb[:, off:off + 128])
                            g = p1.tile([128, 128], BF, name="enT", tag="enT")
                            nc.scalar.activation(g[:], pe[:], AF.Gelu,
                                                 bias=ben_sb[t])
                            enT.append(g)
                        pv = psum(128, KD)
                        for t in range(2):
                            nc.tensor.matmul(pv[:], enT[t][:], wdn_sb[t][:],
                                             start=(t == 0), stop=(t == 1))
                        nc.vector.tensor_copy(
                            v_all[ni][:, b * 33:b * 33 + KD], pv[:])
                        if b == 0:
                            for bb in range(B):
                                nc.vector.memset(
                                    v_all[ni][:, bb * 33 + 32:bb * 33 + 33],
                                    1.0)

                # a_sb[ni] = exp(-c_h * mc) * (mc <= thr)
                a_sb = [p1k.tile([128, M], BF, name=f"as{i}", tag=f"as{i}")
                        for i in range(32)]
                for ni in range(32):
                    mct = p1.tile([128, M], BF, name="mct", tag="mct")
                    nc.sync.dma_start(mct[:],
                                      mc_dram[ni * 128:(ni + 1) * 128, :])
                    msk = p1.tile([128, M], BF, name="msk", tag="msk")
                    nc.vector.tensor_tensor(msk[:], mct[:], thrD[:], OP.is_le)
                    nc.scalar.activation(a_sb[ni][:], mct[:], AF.Exp,
                                         scale=ncd_sb[:])
                    nc.vector.tensor_tensor(a_sb[ni][:], a_sb[ni][:], msk[:],
                                            OP.mult)

                xhT = p1k.tile([128, M], BF, name="xhT", tag="xhT")
                for mi in range(8):
                    px = psum(128, 4 * 33)
                    for ni in range(32):
                        nc.tensor.matmul(
                            px[:], a_sb[ni][:, mi * 128:(mi + 1) * 128],
                            v_all[ni][:], start=(ni == 0), stop=(ni == 31))
                    for b in range(B):
                        rc = small.tile([128, 1], F32, name="rc", tag="rc")
                        nc.vector.reciprocal(
                            rc[:], px[:, b * 33 + 32:b * 33 + 33])
                        gx = small.tile([128, KD], BF, name="gx", tag="gx")
                        nc.scalar.activation(gx[:], px[:, b * 33:b * 33 + KD],
                                             AF.Gelu, scale=rc[:])
                        ptr = ppt.tile([KD, 128], BF, name="tp", tag="tp")
                        nc.tensor.transpose(ptr[:], gx[:], ident[:])
                        nc.vector.tensor_copy(
                            xhT[b * 32:b * 32 + 32,
                                mi * 128:(mi + 1) * 128], ptr[:])
                # core h sends batch c//2's 32 feature rows to core c
                for c in range(NCORE):
                    nc.sync.dma_start(
                        ag1_in[c * 32:(c + 1) * 32, :],
                        xhT[(c // 2) * 32:(c // 2) * 32 + 32, :])
                nc.gpsimd.collective_compute(
                    "AllToAll", OP.bypass, replica_groups=ALL8,
                    ins=[ag1_in.opt()], outs=[ag1_out.opt()])

            # ---------------- P2: blocks ----------------
            # every core computes ALL 8 heads for its batch (pid//2): the
            # extra matmuls are ~free while each PAIRS collective costs ms
            with tc.tile_pool(name="p2", bufs=2) as p2, \
                 tc.tile_pool(name="p2s", bufs=2) as p2s, \
                 tc.tile_pool(name="p2e", bufs=1) as p2e:
                xT = [p2e.tile([128, M], BF, name=f"xT{t}", tag=f"xT{t}")
                      for t in range(2)]
                for hh in range(H):
                    nc.sync.dma_start(
                        xT[hh // 4][(hh % 4) * 32:(hh % 4) * 32 + 32, :],
                        ag1_out[hh * 32:(hh + 1) * 32, :])

                for blk in range(NB):
                    qp_sb, kp_sb, vp_sb = [], [], []
                    for t in range(2):
                        for dst, base in ((qp_sb, SH_QP), (kp_sb, SH_KP),
                                          (vp_sb, SH_VP)):
                            _lwn[0] += 1
                            w = p2.tile([128, 256], BF, name=f"lw{_lwn[0]}",
                                        tag=f"lw{_lwn[0]}")
                            nc.sync.dma_start(
                                w[:], sheet_out[base + t * 128:
                                                base + (t + 1) * 128,
                                                blk * 256:(blk + 1) * 256])
                            dst.append(w)

                    qt = [p2e.tile([128, M], BF, name=f"qt{g}", tag=f"qt{g}")
                          for g in range(2)]
                    kt = [p2e.tile([128, M], BF, name=f"kt{g}", tag=f"kt{g}")
                          for g in range(2)]
                    for dst2, wsb in ((qt, qp_sb), (kt, kp_sb)):
                        for g in range(2):
                            for mh in range(2):
                                pq = psum(128, 512)
                                for t in range(2):
                                    nc.tensor.matmul(
                                        pq[:],
                                        wsb[t][:, g * 128:(g + 1) * 128],
                                        xT[t][:, mh * 512:(mh + 1) * 512],
                                        start=(t == 0), stop=(t == 1))
                                nc.vector.tensor_copy(
                                    dst2[g][:, mh * 512:(mh + 1) * 512],
                                    pq[:])
                    qh = [p2e.tile([KD, M], BF, name=f"qh{h}", tag=f"qh{h}")
                          for h in range(H)]
                    kh = [p2e.tile([KD, M], BF, name=f"kh{h}", tag=f"kh{h}")
                          for h in range(H)]
                    for h in range(H):
                        nc.vector.tensor_copy(
                            qh[h][:], qt[h // 4][(h % 4) * 32:
                                                 (h % 4) * 32 + 32, :])
                        nc.vector.tensor_copy(
                            kh[h][:], kt[h // 4][(h % 4) * 32:
                                                 (h % 4) * 32 + 32, :])

                    vh = [p2e.tile([128, H * 33], BF, name=f"vh{ni}",
                                   tag=f"vh{ni}") for ni in range(8)]
                    for ni in range(8):
                        pvv = psum(128, 256)
                        for t in range(2):
                            nc.tensor.matmul(
                                pvv[:],
                                xT[t][:, ni * 128:(ni + 1) * 128],
                                vp_sb[t][:], start=(t == 0), stop=(t == 1))
                        for h in range(H):
                            nc.vector.tensor_copy(
                                vh[ni][:, h * 33:h * 33 + KD],
                                pvv[:, h * 32:h * 32 + 32])
                            nc.vector.memset(
                                vh[ni][:, h * 33 + 32:h * 33 + 33], 1.0)

                    paT = [p2e.tile([128, M], BF, name=f"paT{t}",
                                    tag=f"paT{t}") for t in range(2)]
                    for h in range(H):
                        es = [p2s.tile([128, M], BF, name=f"es{ni}",
                                       tag=f"es{ni}") for ni in range(8)]
                        for ni in range(8):
                            for mh in range(2):
                                psc = psum(128, 512)
                                nc.tensor.matmul(
                                    psc[:],
                                    kh[h][:, ni * 128:(ni + 1) * 128],
                                    qh[h][:, mh * 512:(mh + 1) * 512])
                                nc.scalar.activation(
                                    es[ni][:, mh * 512:(mh + 1) * 512],
                                    psc[:], AF.Exp, scale=INV_SQRT_K)
                        for mi in range(8):
                            pa = psum(128, 33)
                            for ni in range(8):
                                nc.tensor.matmul(
                                    pa[:],
                                    es[ni][:, mi * 128:(mi + 1) * 128],
                                    vh[ni][:, h * 33:h * 33 + 33],
                                    start=(ni == 0), stop=(ni == 7))
                            rc = small.tile([128, 1], F32, name="rc", tag="rc")
                            nc.vector.reciprocal(rc[:], pa[:, 32:33])
                            gx = small.tile([128, KD], BF, name="gx", tag="gx")
                            nc.scalar.activation(gx[:], pa[:, 0:KD], AF.Gelu,
                                                 scale=rc[:])
                            ptr = ppt.tile([KD, 128], BF, name="tp", tag="tp")
                            nc.tensor.transpose(ptr[:], gx[:], ident[:])
                            nc.vector.tensor_copy(
                                paT[h // 4][(h % 4) * 32:(h % 4) * 32 + 32,
                                            mi * 128:(mi + 1) * 128], ptr[:])

                    w1_sb = [[lw(p2, SH_W1 + i * 128, SH_W1 + (i + 1) * 128,
                                 blk * 256 + o * 128, blk * 256 + (o + 1) * 128)
                              for o in range(2)] for i in range(2)]
                    b1_sb = [blobS_sb[:, SC_B1 + blk * 2 + t:
                                      SC_B1 + blk * 2 + t + 1]
                             for t in range(2)]
                    h1 = [p2e.tile([128, M], BF, name=f"h1{t}", tag=f"h1{t}")
                          for t in range(2)]
                    for o in range(2):
                        for mh in range(2):
                            ph = psum(128, 512)
                            for i in range(2):
                                nc.tensor.matmul(
                                    ph[:], w1_sb[i][o][:],
                                    paT[i][:, mh * 512:(mh + 1) * 512],
                                    start=(i == 0), stop=(i == 1))
                            nc.scalar.activation(
                                h1[o][:, mh * 512:(mh + 1) * 512], ph[:],
                                AF.Gelu, bias=b1_sb[o])

                    w2_sb = [[lw(p2, SH_W2 + i * 128, SH_W2 + (i + 1) * 128,
                                 blk * 256 + o * 128, blk * 256 + (o + 1) * 128)
                              for o in range(2)] for i in range(2)]
                    wr_sb = [[lw(p2, SH_WR + i * 128, SH_WR + (i + 1) * 128,
                                 blk * 256 + o * 128, blk * 256 + (o + 1) * 128)
                              for o in range(2)] for i in range(2)]
                    bc_sb = [blobS_sb[:, SC_BC + blk * 2 + t:
                                      SC_BC + blk * 2 + t + 1]
                             for t in range(2)]
                    xn = [p2e.tile([128, M], BF, name=f"xn{t}", tag=f"xn{t}")
                          for t in range(2)]
                    for o in range(2):
                        for mh in range(2):
                            po = psum(128, 512)
                            nc.tensor.matmul(
                                po[:], w2_sb[0][o][:],
                                h1[0][:, mh * 512:(mh + 1) * 512],
                                start=True, stop=False)
                            nc.tensor.matmul(
                                po[:], w2_sb[1][o][:],
                                h1[1][:, mh * 512:(mh + 1) * 512],
                                start=False, stop=False)
                            nc.tensor.matmul(
                                po[:], wr_sb[0][o][:],
                                xT[0][:, mh * 512:(mh + 1) * 512],
                                start=False, stop=False)
                            nc.tensor.matmul(
                                po[:], wr_sb[1][o][:],
                                xT[1][:, mh * 512:(mh + 1) * 512],
                                start=False, stop=True)
                            nc.scalar.activation(
                                xn[o][:, mh * 512:(mh + 1) * 512], po[:],
                                AF.Gelu, bias=bc_sb[o])
                    xT = xn

                for t in range(2):
                    nc.sync.dma_start(ag3_in[t * 128:(t + 1) * 128, :],
                                      xT[t][:])
                nc.gpsimd.collective_compute(
                    "AllGather", OP.bypass, replica_groups=ALL8,
                    ins=[ag3_in.opt()], outs=[ag3_out.opt()])

            # ---------------- P4: up (head pid over full N) ----------------
            with tc.tile_pool(name="p4", bufs=3) as p4, \
                 tc.tile_pool(name="p4keep", bufs=1) as p4k:
                vu = [p4k.tile([128, 4 * 33], BF, name=f"vu{i}", tag=f"vu{i}")
                      for i in range(8)]
                for b in range(B):
                    xb = [p4.tile([128, M], BF, name=f"xb{t}", tag=f"xb{t}")
                          for t in range(2)]
                    for t in range(2):
                        nc.sync.dma_start(
                            xb[t][:],
                            ag3_out[2 * b * D + t * 128:
                                    2 * b * D + (t + 1) * 128, :])
                    for mi in range(8):
                        pv = psum(128, KD)
                        for t in range(2):
                            nc.tensor.matmul(
                                pv[:], xb[t][:, mi * 128:(mi + 1) * 128],
                                wup_sb[t][:], start=(t == 0), stop=(t == 1))
                        nc.vector.tensor_copy(
                            vu[mi][:, b * 33:b * 33 + KD], pv[:])
                        if b == 0:
                            for bb in range(B):
                                nc.vector.memset(
                                    vu[mi][:, bb * 33 + 32:bb * 33 + 33], 1.0)

                # thrU (128, N) from the gathered t_up rows (matmul broadcast)
                tu_row = p4k.tile([1, N], BF, name="tur", tag="tur")
                for r in range(4):
                    nc.sync.dma_start(
                        tu_row[:, r * 1024:(r + 1) * 1024],
                        blob_out[R_TUP + r:R_TUP + r + 1, :])
                thrU = p4k.tile([128, N], BF, name="thrU", tag="thrU")
                for hf in range(8):
                    pb = pp.tile([128, 512], F32, name="pp", tag="pp")
                    nc.tensor.matmul(pb[:], ones_sb[:],
                                     tu_row[:, hf * 512:(hf + 1) * 512])
                    nc.vector.tensor_copy(thrU[:, hf * 512:(hf + 1) * 512],
                                          pb[:])

                # eu[ki] = exp(-c_h * mcT) * (mcT <= thrU), mcT via DMA transpose
                eu = [p4k.tile([128, N], BF, name=f"eu{i}", tag=f"eu{i}")
                      for i in range(8)]
                for ki in range(8):
                    mctT = p4.tile([128, N], BF, name="mctT", tag="mctT")
                    nc.sync.dma_start(mctT[:],
                                      mc_dram[:, ki * 128:(ki + 1) * 128],
                                      transpose=True)
                    msk = p4.tile([128, N], BF, name="umsk", tag="umsk")
                    nc.vector.tensor_tensor(msk[:], mctT[:], thrU[:], OP.is_le)
                    nc.scalar.activation(eu[ki][:], mctT[:], AF.Exp,
                                         scale=ncu_sb[:])
                    nc.vector.tensor_tensor(eu[ki][:], eu[ki][:], msk[:],
                                            OP.mult)

                deT = p4k.tile([KD, BN], BF, name="deT", tag="deT")
                for qi in range(32):
                    pd = psum(128, 4 * 33)
                    for ki in range(8):
                        nc.tensor.matmul(
                            pd[:], eu[ki][:, qi * 128:(qi + 1) * 128],
                            vu[ki][:], start=(ki == 0), stop=(ki == 7))
                    for b in range(B):
                        rc = small.tile([128, 1], F32, name="rc", tag="rc")
                        nc.vector.reciprocal(
                            rc[:], pd[:, b * 33 + 32:b * 33 + 33])
                        gx = small.tile([128, KD], BF, name="gx", tag="gx")
                        nc.scalar.activation(gx[:], pd[:, b * 33:b * 33 + KD],
                                             AF.Gelu, scale=rc[:])
                        ptr = ppt.tile([KD, 128], BF, name="tp", tag="tp")
                        nc.tensor.transpose(ptr[:], gx[:], ident[:])
                        nc.vector.tensor_copy(
                            deT[:, b * N + qi * 128:b * N + (qi + 1) * 128],
                            ptr[:])
                # core h sends token chunk c (its 32 de^T rows) to core c
                for c in range(NCORE):
                    nc.sync.dma_start(ag4_in[c * 32:(c + 1) * 32, :],
                                      deT[:, c * TS:(c + 1) * TS])
                nc.gpsimd.collective_compute(
                    "AllToAll", OP.bypass, replica_groups=ALL8,
                    ins=[ag4_in.opt()], outs=[ag4_out.opt()])

            # ---------------- P5: decoder on token shard ----------------
            with tc.tile_pool(name="p5", bufs=2) as p5:
                dea = [p5.tile([128, TS], BF, name=f"dea{t}", tag=f"dea{t}")
                       for t in range(2)]
                for t in range(2):
                    nc.sync.dma_start(
                        dea[t][:],
                        ag4_out[t * 128:(t + 1) * 128, :])
                wd1 = [[lw(p5, SH_MISC + i * 128, SH_MISC + (i + 1) * 128,
                           o * 128, (o + 1) * 128) for o in range(2)]
                       for i in range(2)]
                bd1 = [blobS_sb[:, SC_BD1 + t:SC_BD1 + t + 1]
                       for t in range(2)]
                wd2 = [lw(p5, SH_MISC + 8, SH_MISC + 8 + 128,
                          770 + t, 771 + t) for t in range(2)]
                g = [p5.tile([128, TS], BF, name=f"g{t}", tag=f"g{t}")
                     for t in range(2)]
                for o in range(2):
                    for th in range(4):
                        pg = psum(128, 512)
                        for i in range(2):
                            nc.tensor.matmul(
                                pg[:], wd1[i][o][:],
                                dea[i][:, th * 512:(th + 1) * 512],
                                start=(i == 0), stop=(i == 1))
                        nc.scalar.activation(
                            g[o][:, th * 512:(th + 1) * 512], pg[:],
                            AF.Gelu, bias=bd1[o])
                osb = p5.tile([1, TS], F32, name="osb", tag="osb")
                for th in range(4):
                    p2o = psum(1, 512)
                    for i in range(2):
                        nc.tensor.matmul(
                            p2o[:], wd2[i][:],
                            g[i][:, th * 512:(th + 1) * 512],
                            start=(i == 0), stop=(i == 1))
                    nc.vector.tensor_copy(
                        osb[:, th * 512:(th + 1) * 512], p2o[:])
                nc.sync.dma_start(out_shard[:, :], osb[:])

    nc.compile()
    return nc


def _prep_inputs(inputs, m_cross, W_en, b_en, r_down, w_down, q_pa, k_pa,
                 v_pa, W1_mlp, b1_mlp, W2_mlp, b2_mlp, W_res, b_res, r_up,
                 w_up, W_de1, b_de1, W_de2, b_de2, y_mean, y_std):
    f32 = np.float32
    mc = np.asarray(m_cross, f32)
    mcb = mc.astype(NPBF)
    mcbf = mcb.astype(f32)

    # bf16-robust percentile thresholds: max passing bf16 value so the
    # device-side (bf16 mc <= thr) comparison reproduces the f32 mask
    kd_ = int(0.30 * (N - 1))          # 1228
    vkd = np.partition(mc, kd_, axis=0)[kd_, :]               # (M,)
    t_down_dev = np.where(mc <= vkd[None, :], mcbf, -np.inf).max(axis=0)
    ku_ = int(0.30 * (M - 1))          # 306
    vku = np.partition(mc, ku_, axis=1)[:, ku_]               # (N,)
    t_up_dev = np.where(mc <= vku[:, None], mcbf, -np.inf).max(axis=1)

    # encoder input (4, BN), group-major rows of 1024
    gx = np.linspace(0.0, 1.0, RES + 1, dtype=f32)[:-1]
    gxx = np.broadcast_to(gx[:, None], (RES, RES))
    gyy = np.broadcast_to(gx[None, :], (RES, RES))
    encf = np.zeros((4, BN), f32)
    encf[0, :] = np.tile(gxx.reshape(-1), B)
    encf[1, :] = np.tile(gyy.reshape(-1), B)
    encf[2, :] = np.asarray(inputs, f32).reshape(BN)
    enc_cat = encf.reshape(4, 16, 1024).transpose(1, 0, 2).reshape(64, 1024)

    c_down = np.tan(0.25 * np.pi * (1.0 + np.sin(np.asarray(r_down, f32)
                                                 .reshape(H)))).astype(f32)
    c_up = np.tan(0.25 * np.pi * (1.0 + np.sin(np.asarray(r_up, f32)
                                               .reshape(H)))).astype(f32)

    # ---- weight sheet (1792, 1024) ----
    sheet = np.zeros((SHEET_ROWS, 1024), NPBF)

    def cat_blocks(w):  # (NB, D, D) -> (D, NB*D)
        return np.asarray(w, f32).transpose(1, 0, 2).reshape(D, NB * D)

    def cat_heads(w):   # (NB, H, D, K) -> (D, NB*H*K)
        return np.asarray(w, f32).transpose(2, 0, 1, 3).reshape(D, NB * H * KD)

    sheet[SH_W1:SH_W1 + D] = cat_blocks(W1_mlp).astype(NPBF)
    sheet[SH_W2:SH_W2 + D] = cat_blocks(W2_mlp).astype(NPBF)
    sheet[SH_WR:SH_WR + D] = cat_blocks(W_res).astype(NPBF)
    sheet[SH_QP:SH_QP + D] = cat_heads(q_pa).astype(NPBF)
    sheet[SH_KP:SH_KP + D] = cat_heads(k_pa).astype(NPBF)
    sheet[SH_VP:SH_VP + D] = cat_heads(v_pa).astype(NPBF)
    sheet[SH_MISC:SH_MISC + D, 0:256] = np.asarray(W_de1, f32).astype(NPBF)
    sheet[SH_MISC:SH_MISC + D, 256:512] = (
        np.asarray(w_down, f32).transpose(1, 0, 2).reshape(D, H * KD)
        .astype(NPBF))
    sheet[SH_MISC:SH_MISC + D, 512:768] = (
        np.asarray(w_up, f32).transpose(1, 0, 2).reshape(D, H * KD)
        .astype(NPBF))
    wen4 = np.zeros((4, D), f32)
    wen4[:3, :] = np.asarray(W_en, f32)
    sheet[SH_MISC:SH_MISC + 4, 768:1024] = wen4.astype(NPBF)
    ystd = float(np.asarray(y_std, f32))
    ymean = float(np.asarray(y_mean, f32))
    wde2f = (np.asarray(W_de2, f32).reshape(D) * ystd).astype(NPBF)
    sheet[SH_MISC + 8:SH_MISC + 8 + 128, 770] = wde2f[0:128]
    sheet[SH_MISC + 8:SH_MISC + 8 + 128, 771] = wde2f[128:256]
    bde2f = float(np.asarray(b_de2, f32).reshape(-1)[0] * ystd + ymean)

    # ---- f32 constants (128, 36) ----
    bS = np.zeros((128, SCOLS), f32)
    bS[:, SC_NCD:SC_NCD + 8] = -c_down[None, :]
    bS[:, SC_NCU:SC_NCU + 8] = -c_up[None, :]
    ben = np.asarray(b_en, f32).reshape(D)
    bS[:, SC_BEN] = ben[0:128]
    bS[:, SC_BEN + 1] = ben[128:256]
    b1f = np.asarray(b1_mlp, f32).reshape(NB, D)
    bcf = (np.asarray(b2_mlp, f32) + np.asarray(b_res, f32)).reshape(NB, D)
    for blk in range(NB):
        for t in range(2):
            bS[:, SC_B1 + blk * 2 + t] = b1f[blk, t * 128:(t + 1) * 128]
            bS[:, SC_BC + blk * 2 + t] = bcf[blk, t * 128:(t + 1) * 128]
    bd1 = np.asarray(b_de1, f32).reshape(D)
    bS[:, SC_BD1] = bd1[0:128]
    bS[:, SC_BD1 + 1] = bd1[128:256]

    in_maps = []
    for c in range(NCORE):
        blob = np.zeros((BLOB_ROWS, 1024), NPBF)
        blob[0:512] = mcb[c * 512:(c + 1) * 512]
        blob[R_ENC:R_ENC + 8] = enc_cat[c * 8:(c + 1) * 8].astype(NPBF)
        blob[R_TDOWN] = t_down_dev.astype(NPBF)
        blob[R_TUP:R_TUP + 4] = t_up_dev.reshape(4, 1024).astype(NPBF)
        in_maps.append({
            "blob": blob,
            "sheet": sheet[c * (SHEET_ROWS // NCORE):
                           (c + 1) * (SHEET_ROWS // NCORE)],
            "blobS": bS,
        })
    return in_maps, bde2f


def kernel(**inputs):
    if "nc" not in _cache:
        _cache["nc"] = _build()
    nc = _cache["nc"]
    in_maps, bde2f = _prep_inputs(**inputs)
    res = run_bass_kernel_spmd(nc, in_maps, core_ids=list(range(NCORE)))
    shards = [res.results[c]["out_shard"].reshape(-1) + np.float32(bde2f)
              for c in range(NCORE)]
    out = np.concatenate(shards).astype(np.float32)
    return out.reshape(B, RES, RES, 1)


# revision 24
# speedup vs baseline: 2.0690x; 1.2314x over previous
"""Trainium2 Bass kernel for nn_LiteTransformer (sparse_attention).

Sharding (8 cores):
  - position-attention (down & up): by head (core c owns head c)
  - self-attention blocks: core c owns batch c//2, heads c%2*4..+4
  - decoder: token-sharded (2048 tokens per core)

Host->device transfer is the wall-clock bottleneck (axon tunnel ~120MB/s,
~0.2s per tensor latency), so inputs are packed into THREE small tensors
per core (~1.5MB total) instead of replicating the 16MB masked distance
matrices everywhere:
  - blob  (525,1024) bf16: m_cross row-shard + encoder-input slice +
    masked-percentile thresholds; AllGathered on device.
  - sheet (224,1024) bf16: 1/8 shard of all weights; AllGathered on device.
  - blobS (128,36)  f32 : per-head exp scales + biases (replicated).
The masked distance matrices exp(-c*(mc + big*(mc>thr))) are computed
on-device as exp(-c*mc) * (mc <= thr); thresholds are host-refined so the
bf16 comparison reproduces the exact f32 percentile mask.
"""

import numpy as np
import ml_dtypes

import jax
# run_bass_kernel_spmd builds a fresh jit closure per call; persist the XLA
# executable so repeat calls skip the ~0.5s re-compile (NEFF is already
# disk-cached separately).
jax.config.update("jax_compilation_cache_dir", "/tmp/jax_cache_kernel")
jax.config.update("jax_persistent_cache_min_entry_size_bytes", 0)
jax.config.update("jax_persistent_cache_min_compile_time_secs", 0)

import concourse.bass as bass
import concourse.mybir as mybir
import concourse.tile as tile
from concourse import bacc
from concourse.bass import ds
from concourse.bass_utils import (run_bass_kernel_spmd as _lib_run_spmd,
                                  BassKernelResults)
from concourse.masks import make_identity

BF = mybir.dt.bfloat16
F32 = mybir.dt.float32
AF = mybir.ActivationFunctionType
OP = mybir.AluOpType
NPBF = ml_dtypes.bfloat16

B, RES, N, M, H, D, KD, NB = 4, 64, 4096, 1024, 8, 256, 32, 4
BN = B * N
NCORE = 8
INV_SQRT_K = float(1.0 / np.sqrt(np.float32(KD)))
ALL8 = [list(range(NCORE))]
PAIRS = [[0, 1], [2, 3], [4, 5], [6, 7]]

# blob layout (per-core rows, width 1024 bf16); m_cross ships separately as
# uint8 (round(mc*255) — same absolute precision as bf16 on [0,1), half the
# bytes, and exact integer threshold compares)
BLOB_ROWS = 13           # 8 enc + 1 t_down + 4 t_up
R_ENC = 0
R_TDOWN = 8
R_TUP = 9
# sheet layout (global rows, width 1024 bf16)
SHEET_ROWS = 1792        # 224 per core
SH_W1, SH_W2, SH_WR = 0, 256, 512
SH_QP, SH_KP, SH_VP = 768, 1024, 1280
SH_MISC = 1536           # cols 0:256 wde1 | 256:512 wdown | 512:768 wup | 768: misc2
# misc2: rows SH_MISC..+4 cols 768:1024 = wen; wde2 halves at cols 770,771 rows +8..+136
# blobS cols
SC_NCD, SC_NCU, SC_BEN, SC_B1, SC_BC, SC_BD1, SCOLS = 0, 8, 16, 18, 26, 34, 36

_cache = {}
_exec_cache = {}


def run_bass_kernel_spmd(nc, in_maps, core_ids, **kw):
    """Same semantics as bass_utils.run_bass_kernel_spmd for the plain SPMD
    case, but keeps the jitted executable across calls (the library builds a
    fresh closure per call, costing ~0.1s of retrace + cache-deserialize).
    Inputs are still transferred and the NEFF executed on hardware each call.
    """
    n_cores = len(core_ids)
    if kw or list(core_ids) != list(range(n_cores)) or nc.dbg_addr is not None:
        return _lib_run_spmd(nc, in_maps, core_ids=core_ids, **kw)
    ent = _exec_cache.get(id(nc))
    if ent is None:
        from jax.sharding import Mesh, PartitionSpec
        from jax.experimental.shard_map import shard_map
        from concourse.bass2jax import (_bass_exec_p, install_neuronx_cc_hook,
                                        partition_id_tensor)
        install_neuronx_cc_hook()
        pname = (nc.partition_id_tensor.name if nc.partition_id_tensor
                 else None)
        in_names, out_names, out_avals, zero_outs = [], [], [], []
        for alloc in nc.m.functions[0].allocations:
            if not isinstance(alloc, mybir.MemoryLocationSet):
                continue
            name = alloc.memorylocations[0].name
            if alloc.kind == "ExternalInput":
                if name != pname:
                    in_names.append(name)
            elif alloc.kind == "ExternalOutput":
                out_names.append(name)
                shape = tuple(alloc.tensor_shape)
                dtype = mybir.dt.np(alloc.dtype)
                out_avals.append(jax.core.ShapedArray(shape, dtype))
                zero_outs.append(np.zeros(shape, dtype))
        n_params = len(in_names)
        all_names = in_names + out_names + ([pname] if pname else [])

        def _body(*args):
            operands = list(args)
            if pname is not None:
                operands.append(partition_id_tensor())
            outs = _bass_exec_p.bind(
                *operands, out_avals=tuple(out_avals),
                in_names=tuple(all_names), out_names=tuple(out_names),
                lowering_input_output_aliases=(), sim_require_finite=True,
                sim_require_nnan=True, nc=nc)
            return tuple(outs)

        devices = jax.devices()[:n_cores]
        mesh = Mesh(np.asarray(devices), ("core",))
        n_io = n_params + len(out_names)
        sharded = jax.jit(
            shard_map(_body, mesh=mesh,
                      in_specs=(PartitionSpec("core"),) * n_io,
                      out_specs=(PartitionSpec("core"),) * len(out_names),
                      check_rep=False),
            donate_argnums=tuple(range(n_params, n_io)), keep_unused=True)
        ent = (sharded, in_names, n_params, out_names, out_avals, zero_outs)
        _exec_cache[id(nc)] = ent
    sharded, in_names, n_params, out_names, out_avals, zero_outs = ent
    concat_in = [
        np.concatenate([np.asarray(in_maps[c][nm]) for c in range(n_cores)],
                       axis=0) for nm in in_names]
    concat_zeros = [np.zeros((n_cores * z.shape[0], *z.shape[1:]), z.dtype)
                    for z in zero_outs]
    out_arrs = sharded(*concat_in, *concat_zeros)
    results = [
        {nm: np.asarray(out_arrs[i]).reshape(n_cores, *out_avals[i].shape)[c]
         for i, nm in enumerate(out_names)}
        for c in range(n_cores)]
    return BassKernelResults(results=results, instructions_and_trace=None,
                             profile_json=None, exec_time_ns=None)


def _build():
    nc = bacc.Bacc("TRN2", target_bir_lowering=False, debug=False,
                   num_devices=NCORE)

    U8 = mybir.dt.uint8
    mcq = nc.dram_tensor("mcq", [N // NCORE, 1024], U8,
                         kind="ExternalInput").ap()
    blob = nc.dram_tensor("blob", [BLOB_ROWS, 1024], BF,
                          kind="ExternalInput").ap()
    sheet = nc.dram_tensor("sheet", [SHEET_ROWS // NCORE, 1024], BF,
                           kind="ExternalInput").ap()
    blobS = nc.dram_tensor("blobS", [128, SCOLS], F32,
                           kind="ExternalInput").ap()
    out_shard = nc.dram_tensor("out_shard", [1, BN // NCORE], F32,
                               kind="ExternalOutput").ap()

    with tile.TileContext(nc) as tc:
        with (
            tc.tile_pool(name="dram", bufs=1, space="DRAM") as dram,
            tc.tile_pool(name="consts", bufs=1) as consts,
            tc.tile_pool(name="small", bufs=6) as small,
            tc.tile_pool(name="pp", bufs=4, space="PSUM") as pp,
            tc.tile_pool(name="pt", bufs=2, space="PSUM") as ppt,
        ):
            ident = consts.tile([128, 128], BF, name="ident", tag="ident")
            make_identity(nc, ident)
            pid = nc.sync.partition_id()

            # ---- gather the packed inputs across cores ----
            # (collectives cannot read IO tensors; bounce through DRAM scratch)
            mcq_in = dram.tile([N // NCORE, 1024], U8, name="mcqi",
                               tag="mcqi")
            nc.sync.dma_start(mcq_in[:, :], mcq[:, :])
            mcq_out = dram.tile([N, 1024], U8, name="mcqo", tag="mcqo",
                                addr_space="Shared")
            nc.gpsimd.collective_compute(
                "AllGather", OP.bypass, replica_groups=ALL8,
                ins=[mcq_in.opt()], outs=[mcq_out.opt()])
            blob_in = dram.tile([BLOB_ROWS, 1024], BF, name="blobi",
                                tag="blobi")
            nc.sync.dma_start(blob_in[:, :], blob[:, :])
            blob_out = dram.tile([NCORE * BLOB_ROWS, 1024], BF, name="blobo",
                                 tag="blobo", addr_space="Shared")
            nc.gpsimd.collective_compute(
                "AllGather", OP.bypass, replica_groups=ALL8,
                ins=[blob_in.opt()], outs=[blob_out.opt()])
            sheet_in = dram.tile([SHEET_ROWS // NCORE, 1024], BF,
                                 name="sheeti", tag="sheeti")
            nc.sync.dma_start(sheet_in[:, :], sheet[:, :])
            sheet_out = dram.tile([SHEET_ROWS, 1024], BF, name="sheeto",
                                  tag="sheeto", addr_space="Shared")
            nc.gpsimd.collective_compute(
                "AllGather", OP.bypass, replica_groups=ALL8,
                ins=[sheet_in.opt()], outs=[sheet_out.opt()])

            # bf16 m_cross scratch (for P4's transposed reads)
            mc_dram = dram.tile([N, 1024], BF, name="mcd", tag="mcd")

            blobS_sb = consts.tile([128, SCOLS], F32, name="bS", tag="bS")
            nc.sync.dma_start(blobS_sb[:], blobS[:, :])
            ncd_sb = consts.tile([128, 1], F32, name="ncd", tag="ncd")
            nc.sync.dma_start(ncd_sb[:], blobS[0:128, ds(SC_NCD + pid, 1)])
            ncu_sb = consts.tile([128, 1], F32, name="ncu", tag="ncu")
            nc.sync.dma_start(ncu_sb[:], blobS[0:128, ds(SC_NCU + pid, 1)])

            # thresholds broadcast to 128 partitions (ones ⊗ row via matmul)
            ones_sb = consts.tile([1, 128], BF, name="ones", tag="ones")
            nc.vector.memset(ones_sb[:], 1.0)
            td_row = consts.tile([1, 1024], BF, name="tdr", tag="tdr")
            nc.sync.dma_start(td_row[:], blob_out[R_TDOWN:R_TDOWN + 1, :])
            thrD = consts.tile([128, 1024], BF, name="thrD", tag="thrD")
            for hf in range(2):
                pb = pp.tile([128, 512], F32, name="pp", tag="pp")
                nc.tensor.matmul(pb[:], ones_sb[:],
                                 td_row[:, hf * 512:(hf + 1) * 512])
                nc.vector.tensor_copy(thrD[:, hf * 512:(hf + 1) * 512], pb[:])

            wen_sb = consts.tile([4, 256], BF, name="wen", tag="wen")
            nc.sync.dma_start(wen_sb[:], sheet_out[SH_MISC:SH_MISC + 4, 768:1024])
            wdn_sb = []
            wup_sb = []
            for t in range(2):
                w = consts.tile([128, KD], BF, name=f"wdn{t}", tag=f"wdn{t}")
                nc.sync.dma_start(
                    w[:], sheet_out[SH_MISC + t * 128:SH_MISC + (t + 1) * 128,
                                    ds(256 + pid * KD, KD)])
                wdn_sb.append(w)
                w = consts.tile([128, KD], BF, name=f"wupt{t}", tag=f"wupt{t}")
                nc.sync.dma_start(
                    w[:], sheet_out[SH_MISC + t * 128:SH_MISC + (t + 1) * 128,
                                    ds(512 + pid * KD, KD)])
                wup_sb.append(w)
            ben_sb = [blobS_sb[:, SC_BEN + t:SC_BEN + t + 1] for t in range(2)]

            TS = BN // NCORE  # 2048
            # ag1/ag4 use AllToAll (each core only needs its batch / token
            # slice), which shrinks the wire bytes 4-8x vs AllGather.
            ag1_in = dram.tile([NCORE * 32, M], BF, name="ag1i", tag="ag1i")
            ag1_out = dram.tile([NCORE * 32, M], BF, name="ag1o", tag="ag1o")
            ag3_in = dram.tile([D, M], BF, name="ag3i", tag="ag3i")
            ag3_out = dram.tile([NCORE * D, M], BF, name="ag3o", tag="ag3o",
                                addr_space="Shared")
            ag4_in = dram.tile([NCORE * KD, TS], BF, name="ag4i", tag="ag4i")
            ag4_out = dram.tile([NCORE * KD, TS], BF, name="ag4o", tag="ag4o")

            def psum(p, f, dt=F32):
                return pp.tile([p, f], dt, name="pp", tag="pp")

            _lwn = [0]

            def lw(pool, p0, p1, f0, f1, dt=BF):
                # load a (p1-p0, f1-f0) tile from sheet_out
                _lwn[0] += 1
                t = pool.tile([p1 - p0, f1 - f0], dt, name=f"lw{_lwn[0]}",
                              tag=f"lw{_lwn[0]}")
                nc.sync.dma_start(t[:], sheet_out[p0:p1, f0:f1])
                return t

            # ---------------- P1: down (head pid over full N) ----------------
            with tc.tile_pool(name="p1", bufs=3) as p1, \
                 tc.tile_pool(name="p1keep", bufs=1) as p1k:
                # encoder input (4, BN) from the 16 gathered groups
                enc_sb = p1k.tile([4, BN], BF, name="enc_sb", tag="enc_sb")
                for g in range(16):
                    src = (g // 2) * BLOB_ROWS + R_ENC + (g % 2) * 4
                    nc.sync.dma_start(enc_sb[:, g * 1024:(g + 1) * 1024],
                                      blob_out[src:src + 4, :])
                v_all = [p1k.tile([128, 4 * 33], BF, name=f"va{i}", tag=f"va{i}")
                         for i in range(32)]
                for b in range(B):
                    for ni in range(32):
                        off = b * N + ni * 128
                        enT = []
                        for t in range(2):
                            pe = psum(128, 128)
                            nc.tensor.matmul(
                                pe[:], wen_sb[:, t * 128:(t + 1) * 128],
                                enc_sb[:, off:off + 128])
                            g = p1.tile([128, 128], BF, name="enT", tag="enT")
                            nc.scalar.activation(g[:], pe[:], AF.Gelu,
                                                 bias=ben_sb[t])
                            enT.append(g)
                        pv = psum(128, KD)
                        for t in range(2):
                            nc.tensor.matmul(pv[:], enT[t][:], wdn_sb[t][:],
                                             start=(t == 0), stop=(t == 1))
                        nc.vector.tensor_copy(
                            v_all[ni][:, b * 33:b * 33 + KD], pv[:])
                        if b == 0:
                            for bb in range(B):
                                nc.vector.memset(
                                    v_all[ni][:, bb * 33 + 32:bb * 33 + 33],
                                    1.0)

                # a_sb[ni] = exp(-c_h/255 * q) * (q <= thr); also writes the
                # bf16 q values to mc_dram for P4's transposed reads
                a_sb = [p1k.tile([128, M], BF, name=f"as{i}", tag=f"as{i}")
                        for i in range(32)]
                for ni in range(32):
                    mq = p1.tile([128, M], U8, name="mq", tag="mq")
                    nc.sync.dma_start(mq[:],
                                      mcq_out[ni * 128:(ni + 1) * 128, :])
                    mct = p1.tile([128, M], BF, name="mct", tag="mct")
                    nc.vector.tensor_copy(mct[:], mq[:])
                    nc.sync.dma_start(mc_dram[ni * 128:(ni + 1) * 128, :],
                                      mct[:])
                    msk = p1.tile([128, M], BF, name="msk", tag="msk")
                    nc.vector.tensor_tensor(msk[:], mct[:], thrD[:], OP.is_le)
                    nc.scalar.activation(a_sb[ni][:], mct[:], AF.Exp,
                                         scale=ncd_sb[:])
                    nc.vector.tensor_tensor(a_sb[ni][:], a_sb[ni][:], msk[:],
                                            OP.mult)

                xhT = p1k.tile([128, M], BF, name="xhT", tag="xhT")
                for mi in range(8):
                    px = psum(128, 4 * 33)
                    for ni in range(32):
                        nc.tensor.matmul(
                            px[:], a_sb[ni][:, mi * 128:(mi + 1) * 128],
                            v_all[ni][:], start=(ni == 0), stop=(ni == 31))
                    for b in range(B):
                        rc = small.tile([128, 1], F32, name="rc", tag="rc")
                        nc.vector.reciprocal(
                            rc[:], px[:, b * 33 + 32:b * 33 + 33])
                        gx = small.tile([128, KD], BF, name="gx", tag="gx")
                        nc.scalar.activation(gx[:], px[:, b * 33:b * 33 + KD],
                                             AF.Gelu, scale=rc[:])
                        ptr = ppt.tile([KD, 128], BF, name="tp", tag="tp")
                        nc.tensor.transpose(ptr[:], gx[:], ident[:])
                        nc.vector.tensor_copy(
                            xhT[b * 32:b * 32 + 32,
                                mi * 128:(mi + 1) * 128], ptr[:])
                # core h sends batch c//2's 32 feature rows to core c
                for c in range(NCORE):
                    nc.sync.dma_start(
                        ag1_in[c * 32:(c + 1) * 32, :],
                        xhT[(c // 2) * 32:(c // 2) * 32 + 32, :])
                nc.gpsimd.collective_compute(
                    "AllToAll", OP.bypass, replica_groups=ALL8,
                    ins=[ag1_in.opt()], outs=[ag1_out.opt()])

            # ---------------- P2: blocks ----------------
            # every core computes ALL 8 heads for its batch (pid//2): the
            # extra matmuls are ~free while each PAIRS collective costs ms
            with tc.tile_pool(name="p2", bufs=2) as p2, \
                 tc.tile_pool(name="p2s", bufs=2) as p2s, \
                 tc.tile_pool(name="p2e", bufs=1) as p2e:
                xT = [p2e.tile([128, M], BF, name=f"xT{t}", tag=f"xT{t}")
                      for t in range(2)]
                for hh in range(H):
                    nc.sync.dma_start(
                        xT[hh // 4][(hh % 4) * 32:(hh % 4) * 32 + 32, :],
                        ag1_out[hh * 32:(hh + 1) * 32, :])

                for blk in range(NB):
                    qp_sb, kp_sb, vp_sb = [], [], []
                    for t in range(2):
                        for dst, base in ((qp_sb, SH_QP), (kp_sb, SH_KP),
                                          (vp_sb, SH_VP)):
                            _lwn[0] += 1
                            w = p2.tile([128, 256], BF, name=f"lw{_lwn[0]}",
                                        tag=f"lw{_lwn[0]}")
                            nc.sync.dma_start(
                                w[:], sheet_out[base + t * 128:
                                                base + (t + 1) * 128,
                                                blk * 256:(blk + 1) * 256])
                            dst.append(w)

                    qt = [p2e.tile([128, M], BF, name=f"qt{g}", tag=f"qt{g}")
                          for g in range(2)]
                    kt = [p2e.tile([128, M], BF, name=f"kt{g}", tag=f"kt{g}")
                          for g in range(2)]
                    for dst2, wsb in ((qt, qp_sb), (kt, kp_sb)):
                        for g in range(2):
                            for mh in range(2):
                                pq = psum(128, 512)
                                for t in range(2):
                                    nc.tensor.matmul(
                                        pq[:],
                                        wsb[t][:, g * 128:(g + 1) * 128],
                                        xT[t][:, mh * 512:(mh + 1) * 512],
                                        start=(t == 0), stop=(t == 1))
                                nc.vector.tensor_copy(
                                    dst2[g][:, mh * 512:(mh + 1) * 512],
                                    pq[:])
                    qh = [p2e.tile([KD, M], BF, name=f"qh{h}", tag=f"qh{h}")
                          for h in range(H)]
                    kh = [p2e.tile([KD, M], BF, name=f"kh{h}", tag=f"kh{h}")
                          for h in range(H)]
                    for h in range(H):
                        nc.vector.tensor_copy(
                            qh[h][:], qt[h // 4][(h % 4) * 32:
                                                 (h % 4) * 32 + 32, :])
                        nc.vector.tensor_copy(
                            kh[h][:], kt[h // 4][(h % 4) * 32:
                                                 (h % 4) * 32 + 32, :])

                    vh = [p2e.tile([128, H * 33], BF, name=f"vh{ni}",
                                   tag=f"vh{ni}") for ni in range(8)]
                    for ni in range(8):
                        pvv = psum(128, 256)
                        for t in range(2):
                            nc.tensor.matmul(
                                pvv[:],
                                xT[t][:, ni * 128:(ni + 1) * 128],
                                vp_sb[t][:], start=(t == 0), stop=(t == 1))
                        for h in range(H):
                            nc.vector.tensor_copy(
                                vh[ni][:, h * 33:h * 33 + KD],
                                pvv[:, h * 32:h * 32 + 32])
                            nc.vector.memset(
                                vh[ni][:, h * 33 + 32:h * 33 + 33], 1.0)

                    paT = [p2e.tile([128, M], BF, name=f"paT{t}",
                                    tag=f"paT{t}") for t in range(2)]
                    for h in range(H):
                        es = [p2s.tile([128, M], BF, name=f"es{ni}",
                                       tag=f"es{ni}") for ni in range(8)]
                        for ni in range(8):
                            for mh in range(2):
                                psc = psum(128, 512)
                                nc.tensor.matmul(
                                    psc[:],
                                    kh[h][:, ni * 128:(ni + 1) * 128],
                                    qh[h][:, mh * 512:(mh + 1) * 512])
                                nc.scalar.activation(
                                    es[ni][:, mh * 512:(mh + 1) * 512],
                                    psc[:], AF.Exp, scale=INV_SQRT_K)
                        for mi in range(8):
                            pa = psum(128, 33)
                            for ni in range(8):
                                nc.tensor.matmul(
                                    pa[:],
                                    es[ni][:, mi * 128:(mi + 1) * 128],
                                    vh[ni][:, h * 33:h * 33 + 33],
                                    start=(ni == 0), stop=(ni == 7))
                            rc = small.tile([128, 1], F32, name="rc", tag="rc")
                            nc.vector.reciprocal(rc[:], pa[:, 32:33])
                            gx = small.tile([128, KD], BF, name="gx", tag="gx")
                            nc.scalar.activation(gx[:], pa[:, 0:KD], AF.Gelu,
                                                 scale=rc[:])
                            ptr = ppt.tile([KD, 128], BF, name="tp", tag="tp")
                            nc.tensor.transpose(ptr[:], gx[:], ident[:])
                            nc.vector.tensor_copy(
                                paT[h // 4][(h % 4) * 32:(h % 4) * 32 + 32,
                                            mi * 128:(mi + 1) * 128], ptr[:])

                    w1_sb = [[lw(p2, SH_W1 + i * 128, SH_W1 + (i + 1) * 128,
                                 blk * 256 + o * 128, blk * 256 + (o + 1) * 128)
                              for o in range(2)] for i in range(2)]
                    b1_sb = [blobS_sb[:, SC_B1 + blk * 2 + t:
                                      SC_B1 + blk * 2 + t + 1]
                             for t in range(2)]
                    h1 = [p2e.tile([128, M], BF, name=f"h1{t}", tag=f"h1{t}")
                          for t in range(2)]
                    for o in range(2):
                        for mh in range(2):
                            ph = psum(128, 512)
                            for i in range(2):
                                nc.tensor.matmul(
                                    ph[:], w1_sb[i][o][:],
                                    paT[i][:, mh * 512:(mh + 1) * 512],
                                    start=(i == 0), stop=(i == 1))
                            nc.scalar.activation(
                                h1[o][:, mh * 512:(mh + 1) * 512], ph[:],
                                AF.Gelu, bias=b1_sb[o])

                    w2_sb = [[lw(p2, SH_W2 + i * 128, SH_W2 + (i + 1) * 128,
                                 blk * 256 + o * 128, blk * 256 + (o + 1) * 128)
                              for o in range(2)] for i in range(2)]
                    wr_sb = [[lw(p2, SH_WR + i * 128, SH_WR + (i + 1) * 128,
                                 blk * 256 + o * 128, blk * 256 + (o + 1) * 128)
                              for o in range(2)] for i in range(2)]
                    bc_sb = [blobS_sb[:, SC_BC + blk * 2 + t:
                                      SC_BC + blk * 2 + t + 1]
                             for t in range(2)]
                    xn = [p2e.tile([128, M], BF, name=f"xn{t}", tag=f"xn{t}")
                          for t in range(2)]
                    for o in range(2):
                        for mh in range(2):
                            po = psum(128, 512)
                            nc.tensor.matmul(
                                po[:], w2_sb[0][o][:],
                                h1[0][:, mh * 512:(mh + 1) * 512],
                                start=True, stop=False)
                            nc.tensor.matmul(
                                po[:], w2_sb[1][o][:],
                                h1[1][:, mh * 512:(mh + 1) * 512],
                                start=False, stop=False)
                            nc.tensor.matmul(
                                po[:], wr_sb[0][o][:],
                                xT[0][:, mh * 512:(mh + 1) * 512],
                                start=False, stop=False)
                            nc.tensor.matmul(
                                po[:], wr_sb[1][o][:],
                                xT[1][:, mh * 512:(mh + 1) * 512],
                                start=False, stop=True)
                            nc.scalar.activation(
                                xn[o][:, mh * 512:(mh + 1) * 512], po[:],
                                AF.Gelu, bias=bc_sb[o])
                    xT = xn

                for t in range(2):
                    nc.sync.dma_start(ag3_in[t * 128:(t + 1) * 128, :],
                                      xT[t][:])
                nc.gpsimd.collective_compute(
                    "AllGather", OP.bypass, replica_groups=ALL8,
                    ins=[ag3_in.opt()], outs=[ag3_out.opt()])

            # ---------------- P4: up (head pid over full N) ----------------
            with tc.tile_pool(name="p4", bufs=3) as p4, \
                 tc.tile_pool(name="p4keep", bufs=1) as p4k:
                vu = [p4k.tile([128, 4 * 33], BF, name=f"vu{i}", tag=f"vu{i}")
                      for i in range(8)]
                for b in range(B):
                    xb = [p4.tile([128, M], BF, name=f"xb{t}", tag=f"xb{t}")
                          for t in range(2)]
                    for t in range(2):
                        nc.sync.dma_start(
                            xb[t][:],
                            ag3_out[2 * b * D + t * 128:
                                    2 * b * D + (t + 1) * 128, :])
                    for mi in range(8):
                        pv = psum(128, KD)
                        for t in range(2):
                            nc.tensor.matmul(
                                pv[:], xb[t][:, mi * 128:(mi + 1) * 128],
                                wup_sb[t][:], start=(t == 0), stop=(t == 1))
                        nc.vector.tensor_copy(
                            vu[mi][:, b * 33:b * 33 + KD], pv[:])
                        if b == 0:
                            for bb in range(B):
                                nc.vector.memset(
                                    vu[mi][:, bb * 33 + 32:bb * 33 + 33], 1.0)

                # thrU (128, N) from the gathered t_up rows (matmul broadcast)
                tu_row = p4k.tile([1, N], BF, name="tur", tag="tur")
                for r in range(4):
                    nc.sync.dma_start(
                        tu_row[:, r * 1024:(r + 1) * 1024],
                        blob_out[R_TUP + r:R_TUP + r + 1, :])
                thrU = p4k.tile([128, N], BF, name="thrU", tag="thrU")
                for hf in range(8):
                    pb = pp.tile([128, 512], F32, name="pp", tag="pp")
                    nc.tensor.matmul(pb[:], ones_sb[:],
                                     tu_row[:, hf * 512:(hf + 1) * 512])
                    nc.vector.tensor_copy(thrU[:, hf * 512:(hf + 1) * 512],
                                          pb[:])

                # eu[ki] = exp(-c_h * mcT) * (mcT <= thrU), mcT via DMA transpose
                eu = [p4k.tile([128, N], BF, name=f"eu{i}", tag=f"eu{i}")
                      for i in range(8)]
                for ki in range(8):
                    mctT = p4.tile([128, N], BF, name="mctT", tag="mctT")
                    nc.sync.dma_start(mctT[:],
                                      mc_dram[:, ki * 128:(ki + 1) * 128],
                                      transpose=True)
                    msk = p4.tile([128, N], BF, name="umsk", tag="umsk")
                    nc.vector.tensor_tensor(msk[:], mctT[:], thrU[:], OP.is_le)
                    nc.scalar.activation(eu[ki][:], mctT[:], AF.Exp,
                                         scale=ncu_sb[:])
                    nc.vector.tensor_tensor(eu[ki][:], eu[ki][:], msk[:],
                                            OP.mult)

                deT = p4k.tile([KD, BN], BF, name="deT", tag="deT")
                for qi in range(32):
                    pd = psum(128, 4 * 33)
                    for ki in range(8):
                        nc.tensor.matmul(
                            pd[:], eu[ki][:, qi * 128:(qi + 1) * 128],
                            vu[ki][:], start=(ki == 0), stop=(ki == 7))
                    for b in range(B):
                        rc = small.tile([128, 1], F32, name="rc", tag="rc")
                        nc.vector.reciprocal(
                            rc[:], pd[:, b * 33 + 32:b * 33 + 33])
                        gx = small.tile([128, KD], BF, name="gx", tag="gx")
                        nc.scalar.activation(gx[:], pd[:, b * 33:b * 33 + KD],
                                             AF.Gelu, scale=rc[:])
                        ptr = ppt.tile([KD, 128], BF, name="tp", tag="tp")
                        nc.tensor.transpose(ptr[:], gx[:], ident[:])
                        nc.vector.tensor_copy(
                            deT[:, b * N + qi * 128:b * N + (qi + 1) * 128],
                            ptr[:])
                # core h sends token chunk c (its 32 de^T rows) to core c
                for c in range(NCORE):
                    nc.sync.dma_start(ag4_in[c * 32:(c + 1) * 32, :],
                                      deT[:, c * TS:(c + 1) * TS])
                nc.gpsimd.collective_compute(
                    "AllToAll", OP.bypass, replica_groups=ALL8,
                    ins=[ag4_in.opt()], outs=[ag4_out.opt()])

            # ---------------- P5: decoder on token shard ----------------
            with tc.tile_pool(name="p5", bufs=2) as p5:
                dea = [p5.tile([128, TS], BF, name=f"dea{t}", tag=f"dea{t}")
                       for t in range(2)]
                for t in range(2):
                    nc.sync.dma_start(
                        dea[t][:],
                        ag4_out[t * 128:(t + 1) * 128, :])
                wd1 = [[lw(p5, SH_MISC + i * 128, SH_MISC + (i + 1) * 128,
                           o * 128, (o + 1) * 128) for o in range(2)]
                       for i in range(2)]
                bd1 = [blobS_sb[:, SC_BD1 + t:SC_BD1 + t + 1]
                       for t in range(2)]
                wd2 = [lw(p5, SH_MISC + 8, SH_MISC + 8 + 128,
                          770 + t, 771 + t) for t in range(2)]
                g = [p5.tile([128, TS], BF, name=f"g{t}", tag=f"g{t}")
                     for t in range(2)]
                for o in range(2):
                    for th in range(4):
                        pg = psum(128, 512)
                        for i in range(2):
                            nc.tensor.matmul(
                                pg[:], wd1[i][o][:],
                                dea[i][:, th * 512:(th + 1) * 512],
                                start=(i == 0), stop=(i == 1))
                        nc.scalar.activation(
                            g[o][:, th * 512:(th + 1) * 512], pg[:],
                            AF.Gelu, bias=bd1[o])
                osb = p5.tile([1, TS], F32, name="osb", tag="osb")
                for th in range(4):
                    p2o = psum(1, 512)
                    for i in range(2):
                        nc.tensor.matmul(
                            p2o[:], wd2[i][:],
                            g[i][:, th * 512:(th + 1) * 512],
                            start=(i == 0), stop=(i == 1))
                    nc.vector.tensor_copy(
                        osb[:, th * 512:(th + 1) * 512], p2o[:])
                nc.sync.dma_start(out_shard[:, :], osb[:])

    nc.compile()
    return nc


def _prep_inputs(inputs, m_cross, W_en, b_en, r_down, w_down, q_pa, k_pa,
                 v_pa, W1_mlp, b1_mlp, W2_mlp, b2_mlp, W_res, b_res, r_up,
                 w_up, W_de1, b_de1, W_de2, b_de2, y_mean, y_std):
    f32 = np.float32
    mc = np.asarray(m_cross, f32)
    # uint8 quantization: q = round(mc*255); same absolute precision as bf16
    # on [0,1) and exact integer threshold comparisons on device
    mcq8 = np.clip(np.round(mc * 255.0), 0, 255).astype(np.uint8)
    mcqf = mcq8.astype(f32)

    # quantization-robust percentile thresholds: max passing q value so the
    # device-side (q <= thr) comparison reproduces the f32 mask
    kd_ = int(0.30 * (N - 1))          # 1228
    vkd = np.partition(mc, kd_, axis=0)[kd_, :]               # (M,)
    t_down_dev = np.where(mc <= vkd[None, :], mcqf, -np.inf).max(axis=0)
    ku_ = int(0.30 * (M - 1))          # 306
    vku = np.partition(mc, ku_, axis=1)[:, ku_]               # (N,)
    t_up_dev = np.where(mc <= vku[:, None], mcqf, -np.inf).max(axis=1)

    # encoder input (4, BN), group-major rows of 1024
    gx = np.linspace(0.0, 1.0, RES + 1, dtype=f32)[:-1]
    gxx = np.broadcast_to(gx[:, None], (RES, RES))
    gyy = np.broadcast_to(gx[None, :], (RES, RES))
    encf = np.zeros((4, BN), f32)
    encf[0, :] = np.tile(gxx.reshape(-1), B)
    encf[1, :] = np.tile(gyy.reshape(-1), B)
    encf[2, :] = np.asarray(inputs, f32).reshape(BN)
    enc_cat = encf.reshape(4, 16, 1024).transpose(1, 0, 2).reshape(64, 1024)

    c_down = np.tan(0.25 * np.pi * (1.0 + np.sin(np.asarray(r_down, f32)
                                                 .reshape(H)))).astype(f32)
    c_up = np.tan(0.25 * np.pi * (1.0 + np.sin(np.asarray(r_up, f32)
                                               .reshape(H)))).astype(f32)

    # ---- weight sheet (1792, 1024) ----
    sheet = np.zeros((SHEET_ROWS, 1024), NPBF)

    def cat_blocks(w):  # (NB, D, D) -> (D, NB*D)
        return np.asarray(w, f32).transpose(1, 0, 2).reshape(D, NB * D)

    def cat_heads(w):   # (NB, H, D, K) -> (D, NB*H*K)
        return np.asarray(w, f32).transpose(2, 0, 1, 3).reshape(D, NB * H * KD)

    sheet[SH_W1:SH_W1 + D] = cat_blocks(W1_mlp).astype(NPBF)
    sheet[SH_W2:SH_W2 + D] = cat_blocks(W2_mlp).astype(NPBF)
    sheet[SH_WR:SH_WR + D] = cat_blocks(W_res).astype(NPBF)
    sheet[SH_QP:SH_QP + D] = cat_heads(q_pa).astype(NPBF)
    sheet[SH_KP:SH_KP + D] = cat_heads(k_pa).astype(NPBF)
    sheet[SH_VP:SH_VP + D] = cat_heads(v_pa).astype(NPBF)
    sheet[SH_MISC:SH_MISC + D, 0:256] = np.asarray(W_de1, f32).astype(NPBF)
    sheet[SH_MISC:SH_MISC + D, 256:512] = (
        np.asarray(w_down, f32).transpose(1, 0, 2).reshape(D, H * KD)
        .astype(NPBF))
    sheet[SH_MISC:SH_MISC + D, 512:768] = (
        np.asarray(w_up, f32).transpose(1, 0, 2).reshape(D, H * KD)
        .astype(NPBF))
    wen4 = np.zeros((4, D), f32)
    wen4[:3, :] = np.asarray(W_en, f32)
    sheet[SH_MISC:SH_MISC + 4, 768:1024] = wen4.astype(NPBF)
    ystd = float(np.asarray(y_std, f32))
    ymean = float(np.asarray(y_mean, f32))
    wde2f = (np.asarray(W_de2, f32).reshape(D) * ystd).astype(NPBF)
    sheet[SH_MISC + 8:SH_MISC + 8 + 128, 770] = wde2f[0:128]
    sheet[SH_MISC + 8:SH_MISC + 8 + 128, 771] = wde2f[128:256]
    bde2f = float(np.asarray(b_de2, f32).reshape(-1)[0] * ystd + ymean)

    # ---- f32 constants (128, 36) ----  (exp scales absorb the /255)
    bS = np.zeros((128, SCOLS), f32)
    bS[:, SC_NCD:SC_NCD + 8] = -c_down[None, :] / 255.0
    bS[:, SC_NCU:SC_NCU + 8] = -c_up[None, :] / 255.0
    ben = np.asarray(b_en, f32).reshape(D)
    bS[:, SC_BEN] = ben[0:128]
    bS[:, SC_BEN + 1] = ben[128:256]
    b1f = np.asarray(b1_mlp, f32).reshape(NB, D)
    bcf = (np.asarray(b2_mlp, f32) + np.asarray(b_res, f32)).reshape(NB, D)
    for blk in range(NB):
        for t in range(2):
            bS[:, SC_B1 + blk * 2 + t] = b1f[blk, t * 128:(t + 1) * 128]
            bS[:, SC_BC + blk * 2 + t] = bcf[blk, t * 128:(t + 1) * 128]
    bd1 = np.asarray(b_de1, f32).reshape(D)
    bS[:, SC_BD1] = bd1[0:128]
    bS[:, SC_BD1 + 1] = bd1[128:256]

    in_maps = []
    for c in range(NCORE):
        blob = np.zeros((BLOB_ROWS, 1024), NPBF)
        blob[R_ENC:R_ENC + 8] = enc_cat[c * 8:(c + 1) * 8].astype(NPBF)
        blob[R_TDOWN] = t_down_dev.astype(NPBF)
        blob[R_TUP:R_TUP + 4] = t_up_dev.reshape(4, 1024).astype(NPBF)
        in_maps.append({
            "mcq": mcq8[c * (N // NCORE):(c + 1) * (N // NCORE)],
            "blob": blob,
            "sheet": sheet[c * (SHEET_ROWS // NCORE):
                           (c + 1) * (SHEET_ROWS // NCORE)],
            "blobS": bS,
        })
    return in_maps, bde2f


def kernel(**inputs):
    if "nc" not in _cache:
        _cache["nc"] = _build()
    nc = _cache["nc"]
    in_maps, bde2f = _prep_inputs(**inputs)
    res = run_bass_kernel_spmd(nc, in_maps, core_ids=list(range(NCORE)))
    shards = [res.results[c]["out_shard"].reshape(-1) + np.float32(bde2f)
              for c in range(NCORE)]
    out = np.concatenate(shards).astype(np.float32)
    return out.reshape(B, RES, RES, 1)
